# revision 41
# baseline (speedup 1.0000x reference)
"""Multi-head attention Trainium2 kernel (8 NeuronCores, tensor+data parallel).

Problem: B=2, S=2048, H=1024, NH=16 heads, DH=64, causal additive mask.
  qkv = hs @ w_qkv ; per-head scaled-dot-product attention ; out = ctx @ w_out

Sharding: core c owns batch b=c//4 and 4 heads g=(c%4)*4..+4.  Each core
computes Q^T/K^T for its head slice, V in normal [s,d] layout, attention in
transposed-score layout (softmax along the PSUM partition axis, sums via a
ones-column augmented V), and a partial out-projection over its 256 head
features; the host sums the partials per batch.

Schedule (all bf16 on-device, PSUM f32):
 - PSUM plan: scores sp 2x[128,1024] (4 banks) + ctxA/ctxB 1 bank each + a
   2x[128,512] fill pool for qkv/outproj filler chains.  Separating the fill
   pool from sp keeps the scores->exp pipeline at depth 2.
 - Startup: loads share a global ~125-250 GB/s cap, so only two DMA queues
   stream during the critical burst (sync: hT chunk-0 pieces; scalar: wqkv
   pieces, v columns, strip, wo), finest pieces first so the first QKV chain
   starts ~10us in; chunks 1-3 ride the gpsimd queue.  Chunk-0 chains run
   chain-major (qk0, qk2, qk1, qk3, v0-3) across three PSUM pools.
 - Fillers: qkv(sc+1) inside attention(sc) (causal); outproj(0) in qc2,
   outproj(1,2) in qc3, so the PE covers the exp-heavy late chunks.  In full
   mode all qkv precedes attention (every k-loop reads every chunk's K/V).
 - Softmax: denominators via the ones column; reciprocal chains run at high
   priority; GpSimd partition_broadcast spreads them across partitions.
 - Tail: the final 512 rows' out-projection is split by head pair: the
   pair-0 half runs as fillers during pair 1's k-loop (stored as partial
   out_t0); after a 256-column-sliced final norm the pair-1 half streams out
   with copies alternating ACT/DVE (out_t1).  The host adds the partials.

Measured on trn2 (8 cores, NTFF): ~157-163 us vs the 175-177 us baseline,
bf16 PE-stream floor ~113 us + ~14 us fixed preamble/epilogue.
"""

import sys

sys.path.insert(0, "/opt/trn_rl_repo")

import numpy as np

B, S, H, NH = 2, 2048, 1024, 16
DH = H // NH  # 64
N_CORES = 8
HEADS_PER_CORE = 4  # 2 pairs
SC = S // 512  # 4 q/s chunks of 512
KT = S // 128  # 16 k tiles of 128
F_CORE = HEADS_PER_CORE * DH  # 256 out-proj features per core

_CACHE = {}


def _build(mode):
    """Build + schedule the Bass program for `mode` in {"causal", "full"}."""
    import concourse.bass as bass
    import concourse.mybir as mybir
    from concourse import bacc
    from concourse.tile import TileContext

    f32 = mybir.dt.float32
    bf16 = mybir.dt.bfloat16
    EXP = mybir.ActivationFunctionType.Exp

    nc = bacc.Bacc("TRN2", target_bir_lowering=False, debug=False,
                   num_devices=N_CORES)

    # all inputs pre-swizzled to SBUF tile layout on the host
    hT4 = nc.dram_tensor("hT4", [SC, 128, 8 * 512], bf16, kind="ExternalInput").ap()
    # [p, ht, j]; j: [q pair0 (2x64), q pair1, k pair0, k pair1, v (4 heads x 64)]
    wqkv = nc.dram_tensor("wqkv", [128, 8, 6 * 128], bf16, kind="ExternalInput").ap()
    wo = nc.dram_tensor("wo", [128, 2, H], bf16, kind="ExternalInput").ap()
    strip = nc.dram_tensor("strip", [128, 2, 896], bf16, kind="ExternalInput").ap()
    # rows 0 .. 3*512 of the output (blocks 0-2, fully accumulated)
    out = nc.dram_tensor("out", [3 * 512, H], bf16, kind="ExternalOutput").ap()
    # last 512 rows come as two partials, split by head pair; host adds them
    out_t0 = nc.dram_tensor("out_t0", [4, 128, 1024], bf16, kind="ExternalOutput").ap()
    out_t1 = nc.dram_tensor("out_t1", [4, 128, 1024], bf16, kind="ExternalOutput").ap()

    def n_kt(qc):  # k-tiles needed for q chunk qc
        return 4 * qc + 4 if mode == "causal" else KT

    store_rings = []

    with TileContext(nc) as tc:
        with (
            tc.tile_pool(name="consts", bufs=1) as consts,
            tc.tile_pool(name="persist", bufs=1) as persist,
            tc.tile_pool(name="stream", bufs=2) as stream,
            tc.tile_pool(name="epool", bufs=6) as epool,
            tc.tile_pool(name="norm", bufs=2) as norm,
            tc.tile_pool(name="outp", bufs=8) as outp,
            tc.tile_pool(name="spP", bufs=2, space="PSUM") as spP,
            tc.tile_pool(name="ctxP", bufs=1, space="PSUM") as ctxP,
            tc.tile_pool(name="fillP", bufs=2, space="PSUM") as fillP,
        ):
            # ---- tiles ----------------------------------------------------
            wqkv_sb = consts.tile([128, 8, 6 * 128], bf16, tag="wqkv")
            wo_sb = consts.tile([128, 2, H], bf16, tag="wo")
            strip_sb = None
            if mode == "causal":
                strip_sb = consts.tile([128, 2, 896], bf16, tag="strip", name="strip_sb")
            # q^T/k^T blocks per s-chunk: [p(d, 2 heads), jt, 512]
            # jt: 0,1 = q pairs, 2,3 = k pairs
            qkvT = [persist.tile([128, 4, 512], bf16, name=f"qkvT{sc}", tag=f"qkvT{sc}") for sc in range(SC)]
            # ctx^T blocks per q-chunk: [p(f within pair), pair, 512]
            ctxT = [persist.tile([128, 2, 512], bf16, name=f"ctxT{qc}", tag=f"ctxT{qc}") for qc in range(SC)]
            # v in normal layout + ones column: [p(k within tile), head, kt, 65]
            v_all = persist.tile([128, HEADS_PER_CORE, KT, 65], bf16, tag="v_all")
            # staging for the tail pair-1 out-projection partial
            t1_sb = persist.tile([128, 4, 1024], bf16, tag="t1sb")
            warm_sb = consts.tile([128, 512], bf16, tag="warmup")

            def emit_const_loads():
                # softmax-denominator ones column of the augmented V, and the
                # ones row for the reciprocal partition-broadcast matmul
                nc.vector.memset(v_all[:, :, :, 64:65], 1.0)
                # HAM warm-up: ~3.6us of dependency-free matmuls on a memset
                # tile while the first DMA pieces are in flight, so the PE
                # clock is already at 2.4 GHz when real work arrives (the
                # clock gate needs one full 3.4us window of activity)
                nc.vector.memset(warm_sb[:], 0.0)
                wps = fillP.tile([128, 512], f32, tag="fill", name="hamwarm")
                for r in range(17):
                    nc.tensor.matmul(wps, lhsT=warm_sb[0:128, 0:128],
                                     rhs=warm_sb[:, :],
                                     start=(r == 0), stop=(r == 16))
                # loads share a global ~125 GB/s cap, so only the two
                # queues carrying critical startup data stream early (sync:
                # hT pieces; scalar: wqkv qk pieces then v / strip / wo) and
                # everything else queues behind them.  NOTE: small-packet
                # (512B-1KB run) DMAs empirically outrun 2-4KB-run ones here,
                # so the interleaved wqkv layout is kept deliberately.
                for lo, hi in ((0, 1), (1, 2), (2, 4), (4, 6), (6, 8)):
                    nc.scalar.dma_start(out=wqkv_sb[:, lo:hi, 0:512], in_=wqkv[:, lo:hi, 0:512])
                nc.scalar.dma_start(out=wqkv_sb[:, :, 512:768], in_=wqkv[:, :, 512:768])
                if strip_sb is not None:
                    nc.scalar.dma_start(out=strip_sb, in_=strip)

            def load_chunk(sc, eng, pieces=2):
                hT_t = stream.tile([128, 8, 512], bf16, tag="hT", name="hT_t")
                step = 8 // pieces
                for ht in range(0, 8, step):
                    eng.dma_start(out=hT_t[:, ht:ht + step, :],
                                  in_=hT4[sc, :, ht * 512:(ht + step) * 512])
                return hT_t

            # ---- filler units ---------------------------------------------
            def qk_filler(sc, hT_t, jt):
                def f():
                    acc = fillP.tile([128, 512], f32, tag="fill", name="ps")
                    for ht in range(8):
                        nc.tensor.matmul(
                            acc,
                            lhsT=wqkv_sb[:, ht, jt * 128:(jt + 1) * 128],
                            rhs=hT_t[:, ht, :],
                            start=(ht == 0),
                            stop=(ht == 7),
                        )
                    nc.vector.tensor_copy(qkvT[sc][:, jt, :], acc)
                return f

            def v_filler(sc, hT_t, i):
                def f():
                    kt = sc * 4 + i
                    pv = fillP.tile([128, 512], f32, tag="fill", name="pv")
                    for ht in range(8):
                        nc.tensor.matmul(
                            pv[:, 0:256],
                            lhsT=hT_t[:, ht, i * 128:(i + 1) * 128],
                            rhs=wqkv_sb[:, ht, 512:768],
                            start=(ht == 0),
                            stop=(ht == 7),
                        )
                    nc.vector.tensor_copy(v_all[:, :, kt, 0:64], pv[:, 0:256])
                return f

            def next_ring():
                eng = store_rings[0]
                store_rings.append(store_rings.pop(0))
                return eng

            def outproj_filler(blk, i, ec, act_copy=False):
                # full 2-pair unit for blocks 0..2: [128 q rows] x [512 cols]
                def f():
                    st = blk * 4 + i
                    po = fillP.tile([128, 512], f32, tag="fill", name="po")
                    for ft in range(2):
                        nc.tensor.matmul(
                            po,
                            lhsT=ctxT[blk][:, ft, i * 128:(i + 1) * 128],
                            rhs=wo_sb[:, ft, ec * 512:(ec + 1) * 512],
                            start=(ft == 0), stop=(ft == 1),
                        )
                    o_sb = outp.tile([128, 512], bf16, tag="o_sb", name="o_sb")
                    if act_copy:
                        nc.scalar.copy(o_sb, po)
                    else:
                        nc.vector.tensor_copy(o_sb, po)
                    next_ring().dma_start(
                        out=out[st * 128:(st + 1) * 128, ec * 512:(ec + 1) * 512],
                        in_=o_sb)
                return f

            def tail_ft0_filler(blk, i, ec, act_copy=False):
                # pair-0 half-unit of the final block's out-projection,
                # stored as a partial (host adds t0 + t1)
                def f():
                    po = fillP.tile([128, 512], f32, tag="fill", name="pot")
                    nc.tensor.matmul(
                        po,
                        lhsT=ctxT[blk][:, 0, i * 128:(i + 1) * 128],
                        rhs=wo_sb[:, 0, ec * 512:(ec + 1) * 512],
                        start=True, stop=True,
                    )
                    o_sb = outp.tile([128, 512], bf16, tag="o_sb", name="ot_sb")
                    if act_copy:
                        nc.scalar.copy(o_sb, po)
                    else:
                        nc.vector.tensor_copy(o_sb, po)
                    next_ring().dma_start(out=out_t0[i, :, ec * 512:(ec + 1) * 512], in_=o_sb)
                return f

            def qkv_fillers(sc, hT_t):
                return ([qk_filler(sc, hT_t, jt) for jt in range(4)]
                        + [v_filler(sc, hT_t, i) for i in range(4)])

            def emit_attention(qc, fillers, tail_fillers=(), tail_exec=None,
                               tail_warm=(), premade=None):
                # tail_fillers: emitted only during the second pair's k-loop
                # (they depend on pair 0's ctxT of this chunk)
                nkt = n_kt(qc)
                steps = 2 * nkt
                nf = len(fillers)
                due = [(j * steps) // nf for j in range(nf)] if nf else []
                step = 0
                fi = 0
                tfi = 0
                ntf = len(tail_fillers)
                tdue = ([nkt + 1 + (j * (nkt - 2)) // ntf for j in range(ntf)]
                        if ntf else [])
                for pair in range(2):
                    hA, hB = 2 * pair, 2 * pair + 1
                    ctxA = ctxP.tile([65, 512], f32, tag="ctxA", name="ctxA")
                    ctxB = ctxP.tile([65, 512], f32, tag="ctxB", name="ctxB")

                    def emit_av(kt, w0, E):
                        nc.tensor.matmul(
                            ctxA[:, w0:512],
                            lhsT=v_all[:, hA, kt, :],
                            rhs=E[:, 0, w0:512],
                            start=(kt == 0), stop=(kt == nkt - 1),
                        )
                        nc.tensor.matmul(
                            ctxB[:, w0:512],
                            lhsT=v_all[:, hB, kt, :],
                            rhs=E[:, 1, w0:512],
                            start=(kt == 0), stop=(kt == nkt - 1),
                        )

                    # software-pipelined: scores(kt) and exp(kt) issue before
                    # AV(kt-1), so the Scalar engine's exps run back-to-back
                    # and AV never waits on a just-issued exp
                    prev = None
                    for kt in range(nkt):
                        # diagonal tiles only need columns j >= 128*t
                        diag = mode == "causal" and kt >= 4 * qc
                        w0 = 128 * (kt - 4 * qc) if diag else 0
                        if premade is not None and (pair, kt) in premade:
                            # scores+exp for this tile ran during the previous
                            # chunk's boundary; AV can start immediately
                            E = premade[(pair, kt)]
                        else:
                            sp = spP.tile([128, 1024], f32, tag="sp")
                            kblk, ki = qkvT[kt // 4], (kt % 4) * 128
                            # transposed scores, 2 heads row-packed on the PE
                            nc.tensor.matmul(
                                sp[:, w0:512],
                                lhsT=kblk[0:64, 2 + pair, ki:ki + 128],
                                rhs=qkvT[qc][0:64, 0 + pair, w0:512],
                                start=True, stop=True,
                            )
                            nc.tensor.matmul(
                                sp[:, 512 + w0:1024],
                                lhsT=kblk[64:128, 2 + pair, ki:ki + 128],
                                rhs=qkvT[qc][64:128, 0 + pair, w0:512],
                                start=True, stop=True,
                            )
                            E = epool.tile([128, 2, 512], bf16, tag="E")
                            spv = sp[:].rearrange("p (two q) -> p two q", two=2)
                            if kt == 0:
                                # split the first exp per head so AV(kt0, head
                                # A) starts half an exp earlier at boundaries
                                nc.scalar.activation(E[:, 0:1, w0:512], spv[:, 0:1, w0:512], EXP)
                                nc.scalar.activation(E[:, 1:2, w0:512], spv[:, 1:2, w0:512], EXP)
                            else:
                                nc.scalar.activation(E[:, :, w0:512], spv[:, :, w0:512], EXP)
                        if prev is not None:
                            emit_av(*prev)
                        # PE filler absorbs the remaining exp-period slack
                        while fi < nf and due[fi] <= step:
                            fillers[fi]()
                            fi += 1
                        while tfi < ntf and pair == 1 and tdue[tfi] <= nkt + kt:
                            tail_fillers[tfi]()
                            tfi += 1
                        step += 1
                        if diag:
                            # zero the strictly-masked staircase inside the window
                            nc.vector.tensor_mul(
                                E[:, :, w0:512],
                                E[:, :, w0:512],
                                strip_sb[:, :, 384:896 - w0])
                        prev = (kt, w0, E)
                    emit_av(*prev)
                    if tail_exec is not None and pair == 1:
                        for w in tail_warm:
                            w()
                    # normalization: rows 64 hold the softmax denominators.
                    # reciprocal reads PSUM directly; high priority so the
                    # chain doesn't queue behind filler copies
                    if pair == 0:
                        with tc.high_priority(offset=40):
                            rA = norm.tile([1, 512], f32, tag="rA", name="rA")
                            rB = norm.tile([1, 512], f32, tag="rB", name="rB")
                            nc.vector.tensor_copy(rA, ctxA[64:65, :])
                            nc.scalar.copy(rB, ctxB[64:65, :])
                            rr = norm.tile([1, 1024], f32, tag="rr", name="rr")
                            rbc = norm.tile([128, 1024], f32, tag="rbc", name="rbc")
                            nc.vector.reciprocal_approx_fast(out=rr[0:1, 0:512], in_=rA[:])
                            nc.gpsimd.partition_broadcast(rbc[0:64, 0:512], rr[0:1, 0:512], channels=64)
                            nc.vector.reciprocal_approx_fast(out=rr[0:1, 512:1024], in_=rB[:])
                            nc.vector.tensor_mul(ctxT[qc][0:64, pair, :], ctxA[0:64, :], rbc[0:64, 0:512])
                            nc.gpsimd.partition_broadcast(rbc[0:64, 512:1024], rr[0:1, 512:1024], channels=64)
                            nc.vector.tensor_mul(ctxT[qc][64:128, pair, :], ctxB[0:64, :], rbc[0:64, 512:1024])
                    elif tail_exec is None:
                        with tc.high_priority(offset=40):
                            rA = norm.tile([1, 512], f32, tag="rA", name="rA")
                            rB = norm.tile([1, 512], f32, tag="rB", name="rB")
                            nc.vector.tensor_copy(rA, ctxA[64:65, :])
                            nc.scalar.copy(rB, ctxB[64:65, :])
                            rr = norm.tile([1, 1024], f32, tag="rr", name="rr")
                            rbc = norm.tile([128, 1024], f32, tag="rbc", name="rbc")
                            nc.vector.reciprocal_approx_fast(out=rr[0:1, 0:512], in_=rA[:])
                            nc.gpsimd.partition_broadcast(rbc[0:64, 0:512], rr[0:1, 0:512], channels=64)
                            nc.vector.reciprocal_approx_fast(out=rr[0:1, 512:1024], in_=rB[:])
                            nc.vector.tensor_mul(ctxT[qc][0:64, pair, :], ctxA[0:64, :], rbc[0:64, 0:512])
                            nc.gpsimd.partition_broadcast(rbc[0:64, 512:1024], rr[0:1, 512:1024], channels=64)
                            nc.vector.tensor_mul(ctxT[qc][64:128, pair, :], ctxB[0:64, :], rbc[0:64, 512:1024])
                    else:
                        # tail pair-1 norm gates the final out-projection, so
                        # it is sliced by 256 q columns to shorten the chain
                        with tc.high_priority(offset=40):
                            rA = norm.tile([1, 512], f32, tag="rA", name="rA")
                            rB = norm.tile([1, 512], f32, tag="rB", name="rB")
                            rr = norm.tile([1, 1024], f32, tag="rr", name="rr")
                            rbc = norm.tile([128, 1024], f32, tag="rbc", name="rbc")
                            nc.vector.tensor_copy(rA[0:1, 0:256], ctxA[64:65, 0:256])
                            nc.scalar.copy(rB[0:1, 0:256], ctxB[64:65, 0:256])
                            nc.vector.reciprocal_approx_fast(out=rr[0:1, 0:256], in_=rA[0:1, 0:256])
                            nc.gpsimd.partition_broadcast(rbc[0:64, 0:256], rr[0:1, 0:256], channels=64)
                            nc.vector.reciprocal_approx_fast(out=rr[0:1, 512:768], in_=rB[0:1, 0:256])
                            nc.gpsimd.partition_broadcast(rbc[0:64, 512:768], rr[0:1, 512:768], channels=64)
                            nc.vector.tensor_copy(rA[0:1, 256:512], ctxA[64:65, 256:512])
                            nc.scalar.copy(rB[0:1, 256:512], ctxB[64:65, 256:512])
                            nc.vector.tensor_mul(ctxT[qc][0:64, pair, 0:256], ctxA[0:64, 0:256], rbc[0:64, 0:256])
                            nc.vector.tensor_mul(ctxT[qc][64:128, pair, 0:256], ctxB[0:64, 0:256], rbc[0:64, 512:768])
                        if tail_exec is not None:
                            tail_exec(0)
                        with tc.high_priority(offset=40):
                            nc.vector.reciprocal_approx_fast(out=rr[0:1, 256:512], in_=rA[0:1, 256:512])
                            nc.gpsimd.partition_broadcast(rbc[0:64, 256:512], rr[0:1, 256:512], channels=64)
                            nc.vector.reciprocal_approx_fast(out=rr[0:1, 768:1024], in_=rB[0:1, 256:512])
                            nc.gpsimd.partition_broadcast(rbc[0:64, 768:1024], rr[0:1, 768:1024], channels=64)
                            nc.vector.tensor_mul(ctxT[qc][0:64, pair, 256:512], ctxA[0:64, 256:512], rbc[0:64, 256:512])
                            nc.vector.tensor_mul(ctxT[qc][64:128, pair, 256:512], ctxB[0:64, 256:512], rbc[0:64, 768:1024])
                        if tail_exec is not None:
                            tail_exec(1)
                # drain any fillers not yet emitted
                while fi < nf:
                    fillers[fi]()
                    fi += 1
                while tfi < ntf:
                    tail_fillers[tfi]()
                    tfi += 1

            # ---- top-level schedule --------------------------------------
            store_rings[:] = [nc.gpsimd, nc.sync]
            hT_cur = stream.tile([128, 8, 512], bf16, tag="hT", name="hT_t")
            for lo, hi in ((0, 1), (1, 2), (2, 4), (4, 6), (6, 8)):
                nc.sync.dma_start(out=hT_cur[:, lo:hi, :],
                                  in_=hT4[0, :, lo * 512:hi * 512])
            emit_const_loads()
            hT_next1 = load_chunk(1, nc.gpsimd)
            # gate chunks 2-3 (and wo) behind the v-columns arrival so they
            # don't steal load bandwidth from the critical startup burst
            gate_sb = consts.tile([1, 8], bf16, tag="gate")
            nc.gpsimd.tensor_copy(gate_sb[0:1, 0:8], wqkv_sb[0:1, 0, 512:520])
            hT_next2 = load_chunk(2, nc.gpsimd)
            nc.gpsimd.dma_start(out=wo_sb, in_=wo)

            # chunk-0 qkv, chain-major in DMA-arrival order: the qk chains
            # stream the hT/wqkv pieces as they land (the qk1/qk3 re-runs
            # bridge the wait for the v columns), then the v chains.  Chains
            # spread across three PSUM pools so copies pipeline.
            def chain_qk(jt, acc):
                for ht in range(8):
                    nc.tensor.matmul(
                        acc,
                        lhsT=wqkv_sb[:, ht, jt * 128:(jt + 1) * 128],
                        rhs=hT_cur[:, ht, :],
                        start=(ht == 0), stop=(ht == 7),
                    )
                if jt % 2 == 0:
                    nc.vector.tensor_copy(qkvT[0][:, jt, :], acc)
                else:
                    nc.scalar.copy(qkvT[0][:, jt, :], acc)

            def chain_v(i, acc):
                for ht in range(8):
                    nc.tensor.matmul(
                        acc[:, 0:256],
                        lhsT=hT_cur[:, ht, i * 128:(i + 1) * 128],
                        rhs=wqkv_sb[:, ht, 512:768],
                        start=(ht == 0), stop=(ht == 7),
                    )
                if i % 2 == 0:
                    nc.vector.tensor_copy(v_all[:, :, i, 0:64], acc[:, 0:256])
                else:
                    nc.scalar.copy(v_all[:, :, i, 0:64], acc[:, 0:256])

            # qk chains run as a 4-wide wavefront: every 2-ht DMA piece
            # feeds 8 matmuls (~3.4us cold) versus a ~3us piece interval, so
            # the PE never starves while chunk 0 streams in
            wacc = [fillP.tile([128, 512], f32, tag="fill", name="wa0"),
                    fillP.tile([128, 512], f32, tag="fill", name="wa1"),
                    spP.tile([128, 512], f32, tag="sp", name="wa2"),
                    spP.tile([128, 512], f32, tag="sp", name="wa3")]
            worder = (0, 2, 1, 3)
            warm_tags = ["ctxA", "ctxB", "ctxA"]
            for ht in range(8):
                for ci, jt in enumerate(worder):
                    nc.tensor.matmul(
                        wacc[ci],
                        lhsT=wqkv_sb[:, ht, jt * 128:(jt + 1) * 128],
                        rhs=hT_cur[:, ht, :],
                        start=(ht == 0), stop=(ht == 7),
                    )
                if ht in (1, 3, 5):
                    # dummy burst bridges the wait for the next DMA piece so
                    # the HAM stays at full clock through the paced startup
                    # (ctx pool is free until the v chains)
                    dt = ctxP.tile([128, 512], f32, tag=warm_tags[ht // 2],
                                   name=f"hamw{ht}")
                    for r in range(10):
                        nc.tensor.matmul(dt, lhsT=warm_sb[0:128, 0:128],
                                         rhs=warm_sb[:, :],
                                         start=(r == 0), stop=(r == 9))
            for ci, jt in enumerate(worder):
                if ci % 2 == 0:
                    nc.vector.tensor_copy(qkvT[0][:, jt, :], wacc[ci])
                else:
                    nc.scalar.copy(qkvT[0][:, jt, :], wacc[ci])
            chain_v(0, ctxP.tile([128, 512], f32, tag="ctxA", name="wa4"))
            chain_v(1, ctxP.tile([128, 512], f32, tag="ctxB", name="wa5"))
            chain_v(2, fillP.tile([128, 512], f32, tag="fill", name="wa6"))
            chain_v(3, fillP.tile([128, 512], f32, tag="fill", name="wa7"))
            deferred0 = []

            hT_chunks = {0: hT_cur, 1: hT_next1, 2: hT_next2}
            if mode == "causal":
                hT_chunks[3] = load_chunk(3, nc.gpsimd)
                # qc0: rest of qkv(0) + qkv(1); qc1: qkv(2); qc2: qkv(3) +
                # outproj(0); qc3: outproj(1,2) + tail pair-0 preruns
                emit_attention(0, qkv_fillers(1, hT_chunks[1]))
                emit_attention(1, qkv_fillers(2, hT_chunks[2]))
                emit_attention(2, qkv_fillers(3, hT_chunks[3])
                               + [outproj_filler(0, i, ec) for i in range(4) for ec in (0,)])
                # prescore qc3 pair-0 kt0-3 in the boundary window: the PE
                # fills the gap left by qc2's norm chain and the ACT engine
                # gets a 4-tile head start on qc3's exp load
                premade = {}
                for kt in range(4):
                    sp = spP.tile([128, 1024], f32, tag="sp", name="presp")
                    kblk, ki = qkvT[kt // 4], (kt % 4) * 128
                    nc.tensor.matmul(sp[:, 0:512],
                                     lhsT=kblk[0:64, 2, ki:ki + 128],
                                     rhs=qkvT[3][0:64, 0, :],
                                     start=True, stop=True)
                    nc.tensor.matmul(sp[:, 512:1024],
                                     lhsT=kblk[64:128, 2, ki:ki + 128],
                                     rhs=qkvT[3][64:128, 0, :],
                                     start=True, stop=True)
                    Ep = epool.tile([128, 2, 512], bf16, tag="Epre", bufs=4, name=f"Epre{kt}")
                    nc.scalar.activation(
                        Ep[:, :, :],
                        sp[:].rearrange("p (two q) -> p two q", two=2)[:, :, :],
                        EXP)
                    premade[(0, kt)] = Ep
                units = [(i, ec) for i in range(4) for ec in range(2)]
                tail_pre = [tail_ft0_filler(3, i, ec) for (i, ec) in units[:6]]
                op_units = ([outproj_filler(0, i, 1) for i in range(4)]
                            + [outproj_filler(b, i, ec)
                               for b in (1, 2) for i in range(4) for ec in range(2)])
                warm = ([tail_ft0_filler(3, i, ec, act_copy=True) for (i, ec) in units[6:]]
                        + op_units[-2:])

                def tail_exec(s):
                    # ft1 (pair-1) half-units of the final block; copies ride
                    # ACT for slice 0 and DVE for slice 1 so neither engine
                    # serializes the tail
                    for i in (2 * s, 2 * s + 1):
                        for ec in range(2):
                            pool, tag = ((spP, "sp") if (i + ec) % 2 else (fillP, "fill"))
                            po = pool.tile([128, 512], f32, tag=tag, name="pot1")
                            nc.tensor.matmul(
                                po,
                                lhsT=ctxT[3][:, 1, i * 128:(i + 1) * 128],
                                rhs=wo_sb[:, 1, ec * 512:(ec + 1) * 512],
                                start=True, stop=True,
                            )
                            if (i + ec) % 2 == 0:
                                nc.scalar.copy(t1_sb[:, i, ec * 512:(ec + 1) * 512], po)
                            else:
                                nc.vector.tensor_copy(t1_sb[:, i, ec * 512:(ec + 1) * 512], po)
                        next_ring().dma_start(out=out_t1[i], in_=t1_sb[:, i, :])

                emit_attention(3, op_units[:-2],
                               tail_fillers=tail_pre, tail_exec=tail_exec,
                               tail_warm=warm, premade=premade)
            else:
                # full mask: every k-loop reads all chunks' K/V, so ALL
                # qkv chains must complete before attention starts
                hT3 = load_chunk(3, nc.gpsimd)
                for sc, hT_t in ((1, hT_chunks[1]), (2, hT_chunks[2]), (3, hT3)):
                    for f in qkv_fillers(sc, hT_t):
                        f()
                emit_attention(0, [])
                emit_attention(1, [outproj_filler(0, i, ec) for i in range(4) for ec in range(2)])
                emit_attention(2, [outproj_filler(1, i, ec) for i in range(4) for ec in range(2)])
                units = [(i, ec) for i in range(4) for ec in range(2)]
                tail_pre = [tail_ft0_filler(3, i, ec) for (i, ec) in units[:6]]
                op_units = [outproj_filler(2, i, ec) for i in range(4) for ec in range(2)]
                warm = ([tail_ft0_filler(3, i, ec, act_copy=True) for (i, ec) in units[6:]]
                        + op_units[-2:])

                def tail_exec(s):
                    # ft1 (pair-1) half-units of the final block; copies ride
                    # ACT for slice 0 and DVE for slice 1 so neither engine
                    # serializes the tail
                    for i in (2 * s, 2 * s + 1):
                        for ec in range(2):
                            pool, tag = ((spP, "sp") if (i + ec) % 2 else (fillP, "fill"))
                            po = pool.tile([128, 512], f32, tag=tag, name="pot1")
                            nc.tensor.matmul(
                                po,
                                lhsT=ctxT[3][:, 1, i * 128:(i + 1) * 128],
                                rhs=wo_sb[:, 1, ec * 512:(ec + 1) * 512],
                                start=True, stop=True,
                            )
                            if (i + ec) % 2 == 0:
                                nc.scalar.copy(t1_sb[:, i, ec * 512:(ec + 1) * 512], po)
                            else:
                                nc.vector.tensor_copy(t1_sb[:, i, ec * 512:(ec + 1) * 512], po)
                        next_ring().dma_start(out=out_t1[i], in_=t1_sb[:, i, :])

                emit_attention(3, op_units[:-2],
                               tail_fillers=tail_pre, tail_exec=tail_exec,
                               tail_warm=warm)

    nc.compile()
    return nc


def _get_program(mode):
    if mode not in _CACHE:
        _CACHE[mode] = _build(mode)
    return _CACHE[mode]


def _classify_mask(mask):
    """Return "causal", "full", or "generic"."""
    m = mask.reshape(B, S, S)
    tril = np.tril_indices(S)
    if np.all(m == 0.0):
        return "full"
    for b in range(B):
        mb = m[b]
        if not np.all(mb[tril] == 0.0):
            return "generic"
        if not np.all(mb[np.triu_indices(S, k=1)] < -240.0):
            return "generic"
    return "causal"


def _prepare_in_maps(hidden_states, w_qkv, w_out):
    import concourse.mybir as mybir

    bf16 = mybir.dt.np(mybir.dt.bfloat16)

    # strip[i, d, x] = 1.0 iff x >= i + 384 (duplicated along d for head pairs)
    base = (np.arange(896, dtype=np.int32)[None, :] >= (np.arange(128, dtype=np.int32)[:, None] + 384)).astype(np.float32)
    strip = np.ascontiguousarray(np.broadcast_to(base[:, None, :], (128, 2, 896))).astype(bf16)

    # hT4[sc, p, ht*512 + s'] = hs[b, sc*512 + s', ht*128 + p]
    hT4 = [np.ascontiguousarray(
        hidden_states[b].reshape(SC, 512, 8, 128).transpose(0, 3, 2, 1).reshape(SC, 128, 8 * 512)
    ).astype(bf16) for b in range(B)]

    in_maps = []
    for c in range(N_CORES):
        b, g = divmod(c, 4)
        cols = []
        for part in (0, 1):  # q, k column groups of w_qkv (pair-packed)
            for pair in range(2):
                for h in (4 * g + 2 * pair, 4 * g + 2 * pair + 1):
                    cols.append(w_qkv[:, part * H + h * DH: part * H + (h + 1) * DH])
        for h in range(4 * g, 4 * g + 4):  # v columns, head-major
            cols.append(w_qkv[:, 2 * H + h * DH: 2 * H + (h + 1) * DH])
        wqkv_c = np.concatenate(cols, axis=1)
        # fold the 1/sqrt(DH) score scale into the q columns
        wqkv_c = np.ascontiguousarray(wqkv_c)
        wqkv_c[:, 0:256] *= 1.0 / np.sqrt(DH)
        # [p, ht, j] layout
        wqkv_c = wqkv_c.reshape(8, 128, 768).transpose(1, 0, 2)
        wo_c = w_out[g * F_CORE:(g + 1) * F_CORE, :].reshape(2, 128, H).transpose(1, 0, 2)
        in_maps.append({"hT4": hT4[b],
                        "wqkv": np.ascontiguousarray(wqkv_c).astype(bf16),
                        "wo": np.ascontiguousarray(wo_c).astype(bf16),
                        "strip": strip})
    return in_maps


def _run(inputs, trace=False):
    from concourse.bass_utils import run_bass_kernel_spmd

    hidden_states = np.asarray(inputs["hidden_states"], dtype=np.float32)
    mask = np.asarray(inputs["attention_mask"], dtype=np.float32)
    w_qkv = np.asarray(inputs["w_qkv"], dtype=np.float32)
    w_out = np.asarray(inputs["w_out"], dtype=np.float32)

    mode = _classify_mask(mask)
    if mode == "generic":
        return _numpy_reference(hidden_states, mask, w_qkv, w_out), None

    nc = _get_program(mode)
    in_maps = _prepare_in_maps(hidden_states, w_qkv, w_out)
    res = run_bass_kernel_spmd(nc, in_maps, list(range(N_CORES)), trace=trace)
    out = np.zeros((B, S, H), dtype=np.float32)
    for c in range(N_CORES):
        r = res.results[c]
        out[c // 4][:3 * 512] += r["out"].astype(np.float32)
        tail = r["out_t0"].astype(np.float32) + r["out_t1"].astype(np.float32)
        out[c // 4][3 * 512:] += tail.reshape(512, H)
    return out, res


def kernel(**inputs):
    out, _ = _run(inputs, trace=False)
    return out


def kernel_traced(**inputs):
    """Like kernel() but with NTFF profiling; returns (out, BassKernelResults)."""
    return _run(inputs, trace=True)


def _numpy_reference(hidden_states, mask, w_qkv, w_out):
    """Exact fallback for unrecognized masks (slow, chunked numpy)."""
    out = np.zeros((B, S, H), dtype=np.float32)
    m = mask.reshape(B, 1, S, S)
    for b in range(B):
        qkv = hidden_states[b] @ w_qkv  # [S, 3H]
        q = qkv[:, 0:H].reshape(S, NH, DH)
        k = qkv[:, H:2 * H].reshape(S, NH, DH)
        v = qkv[:, 2 * H:].reshape(S, NH, DH)
        ctx = np.zeros((S, NH, DH), dtype=np.float32)
        for h in range(NH):
            s = (q[:, h] @ k[:, h].T) / np.sqrt(DH) + m[b, 0]
            s = s - s.max(axis=-1, keepdims=True)
            e = np.exp(s)
            p = e / e.sum(axis=-1, keepdims=True)
            ctx[:, h] = p @ v[:, h]
        out[b] = ctx.reshape(S, H) @ w_out
    return out


# revision 42
# speedup vs baseline: 1.0980x; 1.0980x over previous
"""Multi-head attention Trainium2 kernel (8 NeuronCores, tensor+data parallel).

Problem: B=2, S=2048, H=1024, NH=16 heads, DH=64, causal additive mask.
  qkv = hs @ w_qkv ; per-head scaled-dot-product attention ; out = ctx @ w_out

Sharding: core c owns batch b=c//4 and 4 heads g=(c%4)*4..+4.  Each core
computes Q^T/K^T for its head slice, V in normal [s,d] layout, attention in
transposed-score layout (softmax along the PSUM partition axis, sums via a
ones-column augmented V), and a partial out-projection over its 256 head
features; the host sums the partials per batch.

Schedule (all bf16 on-device, PSUM f32):
 - PSUM plan: scores sp 2x[128,1024] (4 banks) + ctxA/ctxB 1 bank each + a
   2x[128,512] fill pool for qkv/outproj filler chains.  Separating the fill
   pool from sp keeps the scores->exp pipeline at depth 2.
 - Startup: loads share a global ~125-250 GB/s cap, so only two DMA queues
   stream during the critical burst (sync: hT chunk-0 pieces; scalar: wqkv
   pieces, v columns, strip, wo), finest pieces first so the first QKV chain
   starts ~10us in; chunks 1-3 ride the gpsimd queue.  Chunk-0 chains run
   chain-major (qk0, qk2, qk1, qk3, v0-3) across three PSUM pools.
 - Fillers: qkv(sc+1) inside attention(sc) (causal); outproj(0) in qc2,
   outproj(1,2) in qc3, so the PE covers the exp-heavy late chunks.  In full
   mode all qkv precedes attention (every k-loop reads every chunk's K/V).
 - Softmax: denominators via the ones column; reciprocal chains run at high
   priority; GpSimd partition_broadcast spreads them across partitions.
 - Tail: the final 512 rows' out-projection is split by head pair: the
   pair-0 half runs as fillers during pair 1's k-loop (stored as partial
   out_t0); after a 256-column-sliced final norm the pair-1 half streams out
   with copies alternating ACT/DVE (out_t1).  The host adds the partials.

Measured on trn2 (8 cores, NTFF): ~157-163 us vs the 175-177 us baseline,
bf16 PE-stream floor ~113 us + ~14 us fixed preamble/epilogue.
"""

import sys

sys.path.insert(0, "/opt/trn_rl_repo")

import numpy as np

B, S, H, NH = 2, 2048, 1024, 16
DH = H // NH  # 64
N_CORES = 8
HEADS_PER_CORE = 4  # 2 pairs
SC = S // 512  # 4 q/s chunks of 512
KT = S // 128  # 16 k tiles of 128
F_CORE = HEADS_PER_CORE * DH  # 256 out-proj features per core

_CACHE = {}


def _build(mode):
    """Build + schedule the Bass program for `mode` in {"causal", "full"}."""
    import concourse.bass as bass
    import concourse.mybir as mybir
    from concourse import bacc
    from concourse.tile import TileContext

    f32 = mybir.dt.float32
    bf16 = mybir.dt.bfloat16
    EXP = mybir.ActivationFunctionType.Exp

    nc = bacc.Bacc("TRN2", target_bir_lowering=False, debug=False,
                   num_devices=N_CORES)

    # all inputs pre-swizzled to SBUF tile layout on the host
    hT4 = nc.dram_tensor("hT4", [SC, 128, 8 * 512], bf16, kind="ExternalInput").ap()
    # [p, ht, j]; j: [q pair0 (2x64), q pair1, k pair0, k pair1, v (4 heads x 64)]
    wqkv = nc.dram_tensor("wqkv", [128, 8, 6 * 128], bf16, kind="ExternalInput").ap()
    wo = nc.dram_tensor("wo", [128, 2, H], bf16, kind="ExternalInput").ap()
    strip = nc.dram_tensor("strip", [128, 2, 896], bf16, kind="ExternalInput").ap()
    # rows 0 .. 3*512 of the output (blocks 0-2, fully accumulated)
    out = nc.dram_tensor("out", [3 * 512, H], bf16, kind="ExternalOutput").ap()
    # last 512 rows come as two partials, split by head pair; host adds them
    out_t0 = nc.dram_tensor("out_t0", [4, 128, 1024], bf16, kind="ExternalOutput").ap()
    out_t1 = nc.dram_tensor("out_t1", [4, 128, 1024], bf16, kind="ExternalOutput").ap()

    def n_kt(qc):  # k-tiles needed for q chunk qc
        return 4 * qc + 4 if mode == "causal" else KT

    store_rings = []

    with TileContext(nc) as tc:
        with (
            tc.tile_pool(name="consts", bufs=1) as consts,
            tc.tile_pool(name="persist", bufs=1) as persist,
            tc.tile_pool(name="stream", bufs=2) as stream,
            tc.tile_pool(name="epool", bufs=6) as epool,
            tc.tile_pool(name="norm", bufs=2) as norm,
            tc.tile_pool(name="outp", bufs=8) as outp,
            tc.tile_pool(name="spP", bufs=2, space="PSUM") as spP,
            tc.tile_pool(name="ctxP", bufs=1, space="PSUM") as ctxP,
            tc.tile_pool(name="fillP", bufs=2, space="PSUM") as fillP,
        ):
            # ---- tiles ----------------------------------------------------
            wqkv_sb = consts.tile([128, 8, 6 * 128], bf16, tag="wqkv")
            wo_sb = consts.tile([128, 2, H], bf16, tag="wo")
            strip_sb = None
            if mode == "causal":
                strip_sb = consts.tile([128, 2, 896], bf16, tag="strip", name="strip_sb")
            # q^T/k^T blocks per s-chunk: [p(d, 2 heads), jt, 512]
            # jt: 0,1 = q pairs, 2,3 = k pairs
            qkvT = [persist.tile([128, 4, 512], bf16, name=f"qkvT{sc}", tag=f"qkvT{sc}") for sc in range(SC)]
            # ctx^T blocks per q-chunk: [p(f within pair), pair, 512]
            ctxT = [persist.tile([128, 2, 512], bf16, name=f"ctxT{qc}", tag=f"ctxT{qc}") for qc in range(SC)]
            # v in normal layout + ones column: [p(k within tile), head, kt, 65]
            v_all = persist.tile([128, HEADS_PER_CORE, KT, 65], bf16, tag="v_all")
            # staging for the tail pair-1 out-projection partial
            t1_sb = persist.tile([128, 4, 1024], bf16, tag="t1sb")
            warm_sb = consts.tile([128, 512], bf16, tag="warmup")

            def emit_const_loads():
                # softmax-denominator ones column of the augmented V, and the
                # ones row for the reciprocal partition-broadcast matmul
                nc.vector.memset(v_all[:, :, :, 64:65], 1.0)
                # HAM warm-up: ~3.6us of dependency-free matmuls on a memset
                # tile while the first DMA pieces are in flight, so the PE
                # clock is already at 2.4 GHz when real work arrives (the
                # clock gate needs one full 3.4us window of activity)
                nc.vector.memset(warm_sb[:], 0.0)
                wps = fillP.tile([128, 512], f32, tag="fill", name="hamwarm")
                for r in range(17):
                    nc.tensor.matmul(wps, lhsT=warm_sb[0:128, 0:128],
                                     rhs=warm_sb[:, :],
                                     start=(r == 0), stop=(r == 16))
                # loads share a global ~125 GB/s cap, so only the two
                # queues carrying critical startup data stream early (sync:
                # hT pieces; scalar: wqkv qk pieces then v / strip / wo) and
                # everything else queues behind them.  NOTE: small-packet
                # (512B-1KB run) DMAs empirically outrun 2-4KB-run ones here,
                # so the interleaved wqkv layout is kept deliberately.
                for lo, hi in ((0, 1), (1, 2), (2, 4), (4, 6), (6, 8)):
                    nc.scalar.dma_start(out=wqkv_sb[:, lo:hi, 0:512], in_=wqkv[:, lo:hi, 0:512])
                nc.scalar.dma_start(out=wqkv_sb[:, :, 512:768], in_=wqkv[:, :, 512:768])
                if strip_sb is not None:
                    nc.scalar.dma_start(out=strip_sb, in_=strip)

            def load_chunk(sc, eng, pieces=2):
                hT_t = stream.tile([128, 8, 512], bf16, tag="hT", name="hT_t")
                step = 8 // pieces
                for ht in range(0, 8, step):
                    eng.dma_start(out=hT_t[:, ht:ht + step, :],
                                  in_=hT4[sc, :, ht * 512:(ht + step) * 512])
                return hT_t

            # ---- filler units ---------------------------------------------
            def qk_filler(sc, hT_t, jt):
                def f():
                    acc = fillP.tile([128, 512], f32, tag="fill", name="ps")
                    for ht in range(8):
                        nc.tensor.matmul(
                            acc,
                            lhsT=wqkv_sb[:, ht, jt * 128:(jt + 1) * 128],
                            rhs=hT_t[:, ht, :],
                            start=(ht == 0),
                            stop=(ht == 7),
                        )
                    nc.vector.tensor_copy(qkvT[sc][:, jt, :], acc)
                return f

            def v_filler(sc, hT_t, i):
                def f():
                    kt = sc * 4 + i
                    pv = fillP.tile([128, 512], f32, tag="fill", name="pv")
                    for ht in range(8):
                        nc.tensor.matmul(
                            pv[:, 0:256],
                            lhsT=hT_t[:, ht, i * 128:(i + 1) * 128],
                            rhs=wqkv_sb[:, ht, 512:768],
                            start=(ht == 0),
                            stop=(ht == 7),
                        )
                    nc.vector.tensor_copy(v_all[:, :, kt, 0:64], pv[:, 0:256])
                return f

            def next_ring():
                eng = store_rings[0]
                store_rings.append(store_rings.pop(0))
                return eng

            def outproj_filler(blk, i, ec, act_copy=False):
                # full 2-pair unit for blocks 0..2: [128 q rows] x [512 cols]
                def f():
                    st = blk * 4 + i
                    po = fillP.tile([128, 512], f32, tag="fill", name="po")
                    for ft in range(2):
                        nc.tensor.matmul(
                            po,
                            lhsT=ctxT[blk][:, ft, i * 128:(i + 1) * 128],
                            rhs=wo_sb[:, ft, ec * 512:(ec + 1) * 512],
                            start=(ft == 0), stop=(ft == 1),
                        )
                    o_sb = outp.tile([128, 512], bf16, tag="o_sb", name="o_sb")
                    if act_copy:
                        nc.scalar.copy(o_sb, po)
                    else:
                        nc.vector.tensor_copy(o_sb, po)
                    next_ring().dma_start(
                        out=out[st * 128:(st + 1) * 128, ec * 512:(ec + 1) * 512],
                        in_=o_sb)
                return f

            def tail_ft0_filler(blk, i, ec, act_copy=False):
                # pair-0 half-unit of the final block's out-projection,
                # stored as a partial (host adds t0 + t1)
                def f():
                    po = fillP.tile([128, 512], f32, tag="fill", name="pot")
                    nc.tensor.matmul(
                        po,
                        lhsT=ctxT[blk][:, 0, i * 128:(i + 1) * 128],
                        rhs=wo_sb[:, 0, ec * 512:(ec + 1) * 512],
                        start=True, stop=True,
                    )
                    o_sb = outp.tile([128, 512], bf16, tag="o_sb", name="ot_sb")
                    if act_copy:
                        nc.scalar.copy(o_sb, po)
                    else:
                        nc.vector.tensor_copy(o_sb, po)
                    next_ring().dma_start(out=out_t0[i, :, ec * 512:(ec + 1) * 512], in_=o_sb)
                return f

            def qkv_fillers(sc, hT_t):
                return ([qk_filler(sc, hT_t, jt) for jt in range(4)]
                        + [v_filler(sc, hT_t, i) for i in range(4)])

            def emit_attention(qc, fillers, tail_fillers=(), tail_exec=None,
                               tail_warm=(), premade=None):
                # tail_fillers: emitted only during the second pair's k-loop
                # (they depend on pair 0's ctxT of this chunk)
                nkt = n_kt(qc)
                steps = 2 * nkt
                nf = len(fillers)
                due = [(j * steps) // nf for j in range(nf)] if nf else []
                step = 0
                fi = 0
                tfi = 0
                ntf = len(tail_fillers)
                tdue = ([nkt + 1 + (j * (nkt - 2)) // ntf for j in range(ntf)]
                        if ntf else [])
                for pair in range(2):
                    hA, hB = 2 * pair, 2 * pair + 1
                    ctxA = ctxP.tile([65, 512], f32, tag="ctxA", name="ctxA")
                    ctxB = ctxP.tile([65, 512], f32, tag="ctxB", name="ctxB")

                    def emit_av(kt, w0, E):
                        nc.tensor.matmul(
                            ctxA[:, w0:512],
                            lhsT=v_all[:, hA, kt, :],
                            rhs=E[:, 0, w0:512],
                            start=(kt == 0), stop=(kt == nkt - 1),
                        )
                        nc.tensor.matmul(
                            ctxB[:, w0:512],
                            lhsT=v_all[:, hB, kt, :],
                            rhs=E[:, 1, w0:512],
                            start=(kt == 0), stop=(kt == nkt - 1),
                        )

                    # software-pipelined: scores(kt) and exp(kt) issue before
                    # AV(kt-1), so the Scalar engine's exps run back-to-back
                    # and AV never waits on a just-issued exp
                    prev = None
                    for kt in range(nkt):
                        # diagonal tiles only need columns j >= 128*t
                        diag = mode == "causal" and kt >= 4 * qc
                        w0 = 128 * (kt - 4 * qc) if diag else 0
                        if premade is not None and (pair, kt) in premade:
                            # scores+exp for this tile ran during the previous
                            # chunk's boundary; AV can start immediately
                            E = premade[(pair, kt)]
                        else:
                            sp = spP.tile([128, 1024], f32, tag="sp")
                            kblk, ki = qkvT[kt // 4], (kt % 4) * 128
                            # transposed scores, 2 heads row-packed on the PE
                            nc.tensor.matmul(
                                sp[:, w0:512],
                                lhsT=kblk[0:64, 2 + pair, ki:ki + 128],
                                rhs=qkvT[qc][0:64, 0 + pair, w0:512],
                                start=True, stop=True,
                            )
                            nc.tensor.matmul(
                                sp[:, 512 + w0:1024],
                                lhsT=kblk[64:128, 2 + pair, ki:ki + 128],
                                rhs=qkvT[qc][64:128, 0 + pair, w0:512],
                                start=True, stop=True,
                            )
                            E = epool.tile([128, 2, 512], bf16, tag="E")
                            spv = sp[:].rearrange("p (two q) -> p two q", two=2)
                            if kt == 0:
                                # split the first exp per head so AV(kt0, head
                                # A) starts half an exp earlier at boundaries
                                nc.scalar.activation(E[:, 0:1, w0:512], spv[:, 0:1, w0:512], EXP)
                                nc.scalar.activation(E[:, 1:2, w0:512], spv[:, 1:2, w0:512], EXP)
                            else:
                                nc.scalar.activation(E[:, :, w0:512], spv[:, :, w0:512], EXP)
                        if prev is not None:
                            emit_av(*prev)
                        # PE filler absorbs the remaining exp-period slack
                        while fi < nf and due[fi] <= step:
                            fillers[fi]()
                            fi += 1
                        while tfi < ntf and pair == 1 and tdue[tfi] <= nkt + kt:
                            tail_fillers[tfi]()
                            tfi += 1
                        step += 1
                        if diag:
                            # zero the strictly-masked staircase inside the window
                            nc.vector.tensor_mul(
                                E[:, :, w0:512],
                                E[:, :, w0:512],
                                strip_sb[:, :, 384:896 - w0])
                        prev = (kt, w0, E)
                    emit_av(*prev)
                    if tail_exec is not None and pair == 1:
                        for w in tail_warm:
                            w()
                    # normalization: rows 64 hold the softmax denominators.
                    # reciprocal reads PSUM directly; high priority so the
                    # chain doesn't queue behind filler copies
                    if pair == 0:
                        with tc.high_priority(offset=40):
                            rA = norm.tile([1, 512], f32, tag="rA", name="rA")
                            rB = norm.tile([1, 512], f32, tag="rB", name="rB")
                            nc.vector.tensor_copy(rA, ctxA[64:65, :])
                            nc.scalar.copy(rB, ctxB[64:65, :])
                            rr = norm.tile([1, 1024], f32, tag="rr", name="rr")
                            rbc = norm.tile([128, 1024], f32, tag="rbc", name="rbc")
                            nc.vector.reciprocal_approx_fast(out=rr[0:1, 0:512], in_=rA[:])
                            nc.gpsimd.partition_broadcast(rbc[0:64, 0:512], rr[0:1, 0:512], channels=64)
                            nc.vector.reciprocal_approx_fast(out=rr[0:1, 512:1024], in_=rB[:])
                            nc.vector.tensor_mul(ctxT[qc][0:64, pair, :], ctxA[0:64, :], rbc[0:64, 0:512])
                            nc.gpsimd.partition_broadcast(rbc[0:64, 512:1024], rr[0:1, 512:1024], channels=64)
                            nc.vector.tensor_mul(ctxT[qc][64:128, pair, :], ctxB[0:64, :], rbc[0:64, 512:1024])
                    elif tail_exec is None:
                        with tc.high_priority(offset=40):
                            rA = norm.tile([1, 512], f32, tag="rA", name="rA")
                            rB = norm.tile([1, 512], f32, tag="rB", name="rB")
                            nc.vector.tensor_copy(rA, ctxA[64:65, :])
                            nc.scalar.copy(rB, ctxB[64:65, :])
                            rr = norm.tile([1, 1024], f32, tag="rr", name="rr")
                            rbc = norm.tile([128, 1024], f32, tag="rbc", name="rbc")
                            nc.vector.reciprocal_approx_fast(out=rr[0:1, 0:512], in_=rA[:])
                            nc.gpsimd.partition_broadcast(rbc[0:64, 0:512], rr[0:1, 0:512], channels=64)
                            nc.vector.reciprocal_approx_fast(out=rr[0:1, 512:1024], in_=rB[:])
                            nc.vector.tensor_mul(ctxT[qc][0:64, pair, :], ctxA[0:64, :], rbc[0:64, 0:512])
                            nc.gpsimd.partition_broadcast(rbc[0:64, 512:1024], rr[0:1, 512:1024], channels=64)
                            nc.vector.tensor_mul(ctxT[qc][64:128, pair, :], ctxB[0:64, :], rbc[0:64, 512:1024])
                    else:
                        # tail pair-1 norm gates the final out-projection, so
                        # it is sliced by 256 q columns to shorten the chain
                        with tc.high_priority(offset=40):
                            rA = norm.tile([1, 512], f32, tag="rA", name="rA")
                            rB = norm.tile([1, 512], f32, tag="rB", name="rB")
                            rr = norm.tile([1, 1024], f32, tag="rr", name="rr")
                            rbc = norm.tile([128, 1024], f32, tag="rbc", name="rbc")
                            nc.vector.tensor_copy(rA[0:1, 0:256], ctxA[64:65, 0:256])
                            nc.scalar.copy(rB[0:1, 0:256], ctxB[64:65, 0:256])
                            nc.vector.reciprocal_approx_fast(out=rr[0:1, 0:256], in_=rA[0:1, 0:256])
                            nc.gpsimd.partition_broadcast(rbc[0:64, 0:256], rr[0:1, 0:256], channels=64)
                            nc.vector.reciprocal_approx_fast(out=rr[0:1, 512:768], in_=rB[0:1, 0:256])
                            nc.gpsimd.partition_broadcast(rbc[0:64, 512:768], rr[0:1, 512:768], channels=64)
                            nc.vector.tensor_copy(rA[0:1, 256:512], ctxA[64:65, 256:512])
                            nc.scalar.copy(rB[0:1, 256:512], ctxB[64:65, 256:512])
                            nc.vector.tensor_mul(ctxT[qc][0:64, pair, 0:256], ctxA[0:64, 0:256], rbc[0:64, 0:256])
                            nc.vector.tensor_mul(ctxT[qc][64:128, pair, 0:256], ctxB[0:64, 0:256], rbc[0:64, 512:768])
                        if tail_exec is not None:
                            tail_exec(0)
                        with tc.high_priority(offset=40):
                            nc.vector.reciprocal_approx_fast(out=rr[0:1, 256:512], in_=rA[0:1, 256:512])
                            nc.gpsimd.partition_broadcast(rbc[0:64, 256:512], rr[0:1, 256:512], channels=64)
                            nc.vector.reciprocal_approx_fast(out=rr[0:1, 768:1024], in_=rB[0:1, 256:512])
                            nc.gpsimd.partition_broadcast(rbc[0:64, 768:1024], rr[0:1, 768:1024], channels=64)
                            nc.vector.tensor_mul(ctxT[qc][0:64, pair, 256:512], ctxA[0:64, 256:512], rbc[0:64, 256:512])
                            nc.vector.tensor_mul(ctxT[qc][64:128, pair, 256:512], ctxB[0:64, 256:512], rbc[0:64, 768:1024])
                        if tail_exec is not None:
                            tail_exec(1)
                # drain any fillers not yet emitted
                while fi < nf:
                    fillers[fi]()
                    fi += 1
                while tfi < ntf:
                    tail_fillers[tfi]()
                    tfi += 1

            # ---- top-level schedule --------------------------------------
            store_rings[:] = [nc.gpsimd, nc.sync]
            hT_cur = stream.tile([128, 8, 512], bf16, tag="hT", name="hT_t")
            for lo, hi in ((0, 1), (1, 2), (2, 4), (4, 6), (6, 8)):
                nc.sync.dma_start(out=hT_cur[:, lo:hi, :],
                                  in_=hT4[0, :, lo * 512:hi * 512])
            emit_const_loads()
            hT_next1 = load_chunk(1, nc.gpsimd)
            # gate chunks 2-3 (and wo) behind the v-columns arrival so they
            # don't steal load bandwidth from the critical startup burst
            gate_sb = consts.tile([1, 8], bf16, tag="gate")
            nc.gpsimd.tensor_copy(gate_sb[0:1, 0:8], wqkv_sb[0:1, 0, 512:520])
            hT_next2 = load_chunk(2, nc.gpsimd)
            nc.gpsimd.dma_start(out=wo_sb, in_=wo)

            # chunk-0 qkv, chain-major in DMA-arrival order: the qk chains
            # stream the hT/wqkv pieces as they land (the qk1/qk3 re-runs
            # bridge the wait for the v columns), then the v chains.  Chains
            # spread across three PSUM pools so copies pipeline.
            def chain_qk(jt, acc):
                for ht in range(8):
                    nc.tensor.matmul(
                        acc,
                        lhsT=wqkv_sb[:, ht, jt * 128:(jt + 1) * 128],
                        rhs=hT_cur[:, ht, :],
                        start=(ht == 0), stop=(ht == 7),
                    )
                if jt % 2 == 0:
                    nc.vector.tensor_copy(qkvT[0][:, jt, :], acc)
                else:
                    nc.scalar.copy(qkvT[0][:, jt, :], acc)

            def chain_v(i, acc):
                for ht in range(8):
                    nc.tensor.matmul(
                        acc[:, 0:256],
                        lhsT=hT_cur[:, ht, i * 128:(i + 1) * 128],
                        rhs=wqkv_sb[:, ht, 512:768],
                        start=(ht == 0), stop=(ht == 7),
                    )
                if i % 2 == 0:
                    nc.vector.tensor_copy(v_all[:, :, i, 0:64], acc[:, 0:256])
                else:
                    nc.scalar.copy(v_all[:, :, i, 0:64], acc[:, 0:256])

            # qk chains run as a 4-wide wavefront: every 2-ht DMA piece
            # feeds 8 matmuls (~3.4us cold) versus a ~3us piece interval, so
            # the PE never starves while chunk 0 streams in
            wacc = [fillP.tile([128, 512], f32, tag="fill", name="wa0"),
                    fillP.tile([128, 512], f32, tag="fill", name="wa1"),
                    spP.tile([128, 512], f32, tag="sp", name="wa2"),
                    spP.tile([128, 512], f32, tag="sp", name="wa3")]
            worder = (0, 2, 1, 3)
            for ht in range(8):
                for ci, jt in enumerate(worder):
                    nc.tensor.matmul(
                        wacc[ci],
                        lhsT=wqkv_sb[:, ht, jt * 128:(jt + 1) * 128],
                        rhs=hT_cur[:, ht, :],
                        start=(ht == 0), stop=(ht == 7),
                    )
            for ci, jt in enumerate(worder):
                if ci % 2 == 0:
                    nc.vector.tensor_copy(qkvT[0][:, jt, :], wacc[ci])
                else:
                    nc.scalar.copy(qkvT[0][:, jt, :], wacc[ci])
            chain_v(0, ctxP.tile([128, 512], f32, tag="ctxA", name="wa4"))
            chain_v(1, ctxP.tile([128, 512], f32, tag="ctxB", name="wa5"))
            chain_v(2, fillP.tile([128, 512], f32, tag="fill", name="wa6"))
            chain_v(3, fillP.tile([128, 512], f32, tag="fill", name="wa7"))
            deferred0 = []

            hT_chunks = {0: hT_cur, 1: hT_next1, 2: hT_next2}
            if mode == "causal":
                hT_chunks[3] = load_chunk(3, nc.gpsimd)
                # qc0: rest of qkv(0) + qkv(1); qc1: qkv(2); qc2: qkv(3) +
                # outproj(0); qc3: outproj(1,2) + tail pair-0 preruns
                emit_attention(0, qkv_fillers(1, hT_chunks[1]))
                emit_attention(1, qkv_fillers(2, hT_chunks[2]))
                emit_attention(2, qkv_fillers(3, hT_chunks[3])
                               + [outproj_filler(0, i, ec) for i in range(4) for ec in (0,)])
                # prescore qc3 pair-0 kt0-3 in the boundary window: the PE
                # fills the gap left by qc2's norm chain and the ACT engine
                # gets a 4-tile head start on qc3's exp load
                premade = {}
                for kt in range(4):
                    sp = spP.tile([128, 1024], f32, tag="sp", name="presp")
                    kblk, ki = qkvT[kt // 4], (kt % 4) * 128
                    nc.tensor.matmul(sp[:, 0:512],
                                     lhsT=kblk[0:64, 2, ki:ki + 128],
                                     rhs=qkvT[3][0:64, 0, :],
                                     start=True, stop=True)
                    nc.tensor.matmul(sp[:, 512:1024],
                                     lhsT=kblk[64:128, 2, ki:ki + 128],
                                     rhs=qkvT[3][64:128, 0, :],
                                     start=True, stop=True)
                    Ep = epool.tile([128, 2, 512], bf16, tag="Epre", bufs=4, name=f"Epre{kt}")
                    nc.scalar.activation(
                        Ep[:, :, :],
                        sp[:].rearrange("p (two q) -> p two q", two=2)[:, :, :],
                        EXP)
                    premade[(0, kt)] = Ep
                units = [(i, ec) for i in range(4) for ec in range(2)]
                tail_pre = [tail_ft0_filler(3, i, ec) for (i, ec) in units[:6]]
                op_units = ([outproj_filler(0, i, 1) for i in range(4)]
                            + [outproj_filler(b, i, ec)
                               for b in (1, 2) for i in range(4) for ec in range(2)])
                warm = ([tail_ft0_filler(3, i, ec, act_copy=True) for (i, ec) in units[6:]]
                        + op_units[-2:])

                def tail_exec(s):
                    # ft1 (pair-1) half-units of the final block; copies ride
                    # ACT for slice 0 and DVE for slice 1 so neither engine
                    # serializes the tail
                    for i in (2 * s, 2 * s + 1):
                        for ec in range(2):
                            pool, tag = ((spP, "sp") if (i + ec) % 2 else (fillP, "fill"))
                            po = pool.tile([128, 512], f32, tag=tag, name="pot1")
                            nc.tensor.matmul(
                                po,
                                lhsT=ctxT[3][:, 1, i * 128:(i + 1) * 128],
                                rhs=wo_sb[:, 1, ec * 512:(ec + 1) * 512],
                                start=True, stop=True,
                            )
                            if (i + ec) % 2 == 0:
                                nc.scalar.copy(t1_sb[:, i, ec * 512:(ec + 1) * 512], po)
                            else:
                                nc.vector.tensor_copy(t1_sb[:, i, ec * 512:(ec + 1) * 512], po)
                        next_ring().dma_start(out=out_t1[i], in_=t1_sb[:, i, :])

                emit_attention(3, op_units[:-2],
                               tail_fillers=tail_pre, tail_exec=tail_exec,
                               tail_warm=warm, premade=premade)
            else:
                # full mask: every k-loop reads all chunks' K/V, so ALL
                # qkv chains must complete before attention starts
                hT3 = load_chunk(3, nc.gpsimd)
                for sc, hT_t in ((1, hT_chunks[1]), (2, hT_chunks[2]), (3, hT3)):
                    for f in qkv_fillers(sc, hT_t):
                        f()
                emit_attention(0, [])
                emit_attention(1, [outproj_filler(0, i, ec) for i in range(4) for ec in range(2)])
                emit_attention(2, [outproj_filler(1, i, ec) for i in range(4) for ec in range(2)])
                units = [(i, ec) for i in range(4) for ec in range(2)]
                tail_pre = [tail_ft0_filler(3, i, ec) for (i, ec) in units[:6]]
                op_units = [outproj_filler(2, i, ec) for i in range(4) for ec in range(2)]
                warm = ([tail_ft0_filler(3, i, ec, act_copy=True) for (i, ec) in units[6:]]
                        + op_units[-2:])

                def tail_exec(s):
                    # ft1 (pair-1) half-units of the final block; copies ride
                    # ACT for slice 0 and DVE for slice 1 so neither engine
                    # serializes the tail
                    for i in (2 * s, 2 * s + 1):
                        for ec in range(2):
                            pool, tag = ((spP, "sp") if (i + ec) % 2 else (fillP, "fill"))
                            po = pool.tile([128, 512], f32, tag=tag, name="pot1")
                            nc.tensor.matmul(
                                po,
                                lhsT=ctxT[3][:, 1, i * 128:(i + 1) * 128],
                                rhs=wo_sb[:, 1, ec * 512:(ec + 1) * 512],
                                start=True, stop=True,
                            )
                            if (i + ec) % 2 == 0:
                                nc.scalar.copy(t1_sb[:, i, ec * 512:(ec + 1) * 512], po)
                            else:
                                nc.vector.tensor_copy(t1_sb[:, i, ec * 512:(ec + 1) * 512], po)
                        next_ring().dma_start(out=out_t1[i], in_=t1_sb[:, i, :])

                emit_attention(3, op_units[:-2],
                               tail_fillers=tail_pre, tail_exec=tail_exec,
                               tail_warm=warm)

    nc.compile()
    return nc


def _get_program(mode):
    if mode not in _CACHE:
        _CACHE[mode] = _build(mode)
    return _CACHE[mode]


def _classify_mask(mask):
    """Return "causal", "full", or "generic"."""
    m = mask.reshape(B, S, S)
    tril = np.tril_indices(S)
    if np.all(m == 0.0):
        return "full"
    for b in range(B):
        mb = m[b]
        if not np.all(mb[tril] == 0.0):
            return "generic"
        if not np.all(mb[np.triu_indices(S, k=1)] < -240.0):
            return "generic"
    return "causal"


def _prepare_in_maps(hidden_states, w_qkv, w_out):
    import concourse.mybir as mybir

    bf16 = mybir.dt.np(mybir.dt.bfloat16)

    # strip[i, d, x] = 1.0 iff x >= i + 384 (duplicated along d for head pairs)
    base = (np.arange(896, dtype=np.int32)[None, :] >= (np.arange(128, dtype=np.int32)[:, None] + 384)).astype(np.float32)
    strip = np.ascontiguousarray(np.broadcast_to(base[:, None, :], (128, 2, 896))).astype(bf16)

    # hT4[sc, p, ht*512 + s'] = hs[b, sc*512 + s', ht*128 + p]
    hT4 = [np.ascontiguousarray(
        hidden_states[b].reshape(SC, 512, 8, 128).transpose(0, 3, 2, 1).reshape(SC, 128, 8 * 512)
    ).astype(bf16) for b in range(B)]

    in_maps = []
    for c in range(N_CORES):
        b, g = divmod(c, 4)
        cols = []
        for part in (0, 1):  # q, k column groups of w_qkv (pair-packed)
            for pair in range(2):
                for h in (4 * g + 2 * pair, 4 * g + 2 * pair + 1):
                    cols.append(w_qkv[:, part * H + h * DH: part * H + (h + 1) * DH])
        for h in range(4 * g, 4 * g + 4):  # v columns, head-major
            cols.append(w_qkv[:, 2 * H + h * DH: 2 * H + (h + 1) * DH])
        wqkv_c = np.concatenate(cols, axis=1)
        # fold the 1/sqrt(DH) score scale into the q columns
        wqkv_c = np.ascontiguousarray(wqkv_c)
        wqkv_c[:, 0:256] *= 1.0 / np.sqrt(DH)
        # [p, ht, j] layout
        wqkv_c = wqkv_c.reshape(8, 128, 768).transpose(1, 0, 2)
        wo_c = w_out[g * F_CORE:(g + 1) * F_CORE, :].reshape(2, 128, H).transpose(1, 0, 2)
        in_maps.append({"hT4": hT4[b],
                        "wqkv": np.ascontiguousarray(wqkv_c).astype(bf16),
                        "wo": np.ascontiguousarray(wo_c).astype(bf16),
                        "strip": strip})
    return in_maps


def _run(inputs, trace=False):
    from concourse.bass_utils import run_bass_kernel_spmd

    hidden_states = np.asarray(inputs["hidden_states"], dtype=np.float32)
    mask = np.asarray(inputs["attention_mask"], dtype=np.float32)
    w_qkv = np.asarray(inputs["w_qkv"], dtype=np.float32)
    w_out = np.asarray(inputs["w_out"], dtype=np.float32)

    mode = _classify_mask(mask)
    if mode == "generic":
        return _numpy_reference(hidden_states, mask, w_qkv, w_out), None

    nc = _get_program(mode)
    in_maps = _prepare_in_maps(hidden_states, w_qkv, w_out)
    res = run_bass_kernel_spmd(nc, in_maps, list(range(N_CORES)), trace=trace)
    out = np.zeros((B, S, H), dtype=np.float32)
    for c in range(N_CORES):
        r = res.results[c]
        out[c // 4][:3 * 512] += r["out"].astype(np.float32)
        tail = r["out_t0"].astype(np.float32) + r["out_t1"].astype(np.float32)
        out[c // 4][3 * 512:] += tail.reshape(512, H)
    return out, res


def kernel(**inputs):
    out, _ = _run(inputs, trace=False)
    return out


def kernel_traced(**inputs):
    """Like kernel() but with NTFF profiling; returns (out, BassKernelResults)."""
    return _run(inputs, trace=True)


def _numpy_reference(hidden_states, mask, w_qkv, w_out):
    """Exact fallback for unrecognized masks (slow, chunked numpy)."""
    out = np.zeros((B, S, H), dtype=np.float32)
    m = mask.reshape(B, 1, S, S)
    for b in range(B):
        qkv = hidden_states[b] @ w_qkv  # [S, 3H]
        q = qkv[:, 0:H].reshape(S, NH, DH)
        k = qkv[:, H:2 * H].reshape(S, NH, DH)
        v = qkv[:, 2 * H:].reshape(S, NH, DH)
        ctx = np.zeros((S, NH, DH), dtype=np.float32)
        for h in range(NH):
            s = (q[:, h] @ k[:, h].T) / np.sqrt(DH) + m[b, 0]
            s = s - s.max(axis=-1, keepdims=True)
            e = np.exp(s)
            p = e / e.sum(axis=-1, keepdims=True)
            ctx[:, h] = p @ v[:, h]
        out[b] = ctx.reshape(S, H) @ w_out
    return out


# revision 43
# speedup vs baseline: 1.1175x; 1.0178x over previous
"""Multi-head attention Trainium2 kernel (8 NeuronCores, tensor+data parallel).

Problem: B=2, S=2048, H=1024, NH=16 heads, DH=64, causal additive mask.
  qkv = hs @ w_qkv ; per-head scaled-dot-product attention ; out = ctx @ w_out

Sharding: core c owns batch b=c//4 and 4 heads g=(c%4)*4..+4.  Each core
computes Q^T/K^T for its head slice, V in normal [s,d] layout, attention in
transposed-score layout (softmax along the PSUM partition axis, sums via a
ones-column augmented V), and a partial out-projection over its 256 head
features; the host sums the partials per batch.

Schedule (all bf16 on-device, PSUM f32):
 - PSUM plan: scores sp 2x[128,1024] (4 banks) + ctxA/ctxB 1 bank each + a
   2x[128,512] fill pool for qkv/outproj filler chains.  Separating the fill
   pool from sp keeps the scores->exp pipeline at depth 2.
 - Startup: loads share a global ~125-250 GB/s cap, so only two DMA queues
   stream during the critical burst (sync: hT chunk-0 pieces; scalar: wqkv
   pieces, v columns, strip, wo), finest pieces first so the first QKV chain
   starts ~10us in; chunks 1-3 ride the gpsimd queue.  Chunk-0 chains run
   chain-major (qk0, qk2, qk1, qk3, v0-3) across three PSUM pools.
 - Fillers: qkv(sc+1) inside attention(sc) (causal); outproj(0) in qc2,
   outproj(1,2) in qc3, so the PE covers the exp-heavy late chunks.  In full
   mode all qkv precedes attention (every k-loop reads every chunk's K/V).
 - Softmax: denominators via the ones column; reciprocal chains run at high
   priority; GpSimd partition_broadcast spreads them across partitions.
 - Tail: the final 512 rows' out-projection is split by head pair: the
   pair-0 half runs as fillers during pair 1's k-loop (stored as partial
   out_t0); after a 256-column-sliced final norm the pair-1 half streams out
   with copies alternating ACT/DVE (out_t1).  The host adds the partials.

Measured on trn2 (8 cores, NTFF): ~157-163 us vs the 175-177 us baseline,
bf16 PE-stream floor ~113 us + ~14 us fixed preamble/epilogue.
"""

import sys

sys.path.insert(0, "/opt/trn_rl_repo")

import numpy as np

B, S, H, NH = 2, 2048, 1024, 16
DH = H // NH  # 64
N_CORES = 8
HEADS_PER_CORE = 4  # 2 pairs
SC = S // 512  # 4 q/s chunks of 512
KT = S // 128  # 16 k tiles of 128
F_CORE = HEADS_PER_CORE * DH  # 256 out-proj features per core

_CACHE = {}


def _build(mode):
    """Build + schedule the Bass program for `mode` in {"causal", "full"}."""
    import concourse.bass as bass
    import concourse.mybir as mybir
    from concourse import bacc
    from concourse.tile import TileContext

    f32 = mybir.dt.float32
    bf16 = mybir.dt.bfloat16
    EXP = mybir.ActivationFunctionType.Exp

    nc = bacc.Bacc("TRN2", target_bir_lowering=False, debug=False,
                   num_devices=N_CORES)

    # all inputs pre-swizzled to SBUF tile layout on the host
    hT4 = nc.dram_tensor("hT4", [SC, 128, 8 * 512], bf16, kind="ExternalInput").ap()
    # [p, ht, j]; j: [q pair0 (2x64), q pair1, k pair0, k pair1, v (4 heads x 64)]
    wqkv = nc.dram_tensor("wqkv", [128, 8, 6 * 128], bf16, kind="ExternalInput").ap()
    wo = nc.dram_tensor("wo", [128, 2, H], bf16, kind="ExternalInput").ap()
    strip = nc.dram_tensor("strip", [128, 2, 896], bf16, kind="ExternalInput").ap()
    # rows 0 .. 3*512 of the output (blocks 0-2, fully accumulated)
    out = nc.dram_tensor("out", [3 * 512, H], bf16, kind="ExternalOutput").ap()
    # last 512 rows come as two partials, split by head pair; host adds them
    out_t0 = nc.dram_tensor("out_t0", [4, 128, 1024], bf16, kind="ExternalOutput").ap()
    out_t1 = nc.dram_tensor("out_t1", [4, 128, 1024], bf16, kind="ExternalOutput").ap()

    def n_kt(qc):  # k-tiles needed for q chunk qc
        return 4 * qc + 4 if mode == "causal" else KT

    store_rings = []

    with TileContext(nc) as tc:
        with (
            tc.tile_pool(name="consts", bufs=1) as consts,
            tc.tile_pool(name="persist", bufs=1) as persist,
            tc.tile_pool(name="stream", bufs=2) as stream,
            tc.tile_pool(name="epool", bufs=6) as epool,
            tc.tile_pool(name="norm", bufs=2) as norm,
            tc.tile_pool(name="outp", bufs=8) as outp,
            tc.tile_pool(name="spP", bufs=2, space="PSUM") as spP,
            tc.tile_pool(name="ctxP", bufs=1, space="PSUM") as ctxP,
            tc.tile_pool(name="fillP", bufs=2, space="PSUM") as fillP,
        ):
            # ---- tiles ----------------------------------------------------
            wqkv_sb = consts.tile([128, 8, 6 * 128], bf16, tag="wqkv")
            wo_sb = consts.tile([128, 2, H], bf16, tag="wo")
            strip_sb = None
            if mode == "causal":
                strip_sb = consts.tile([128, 2, 896], bf16, tag="strip", name="strip_sb")
            # q^T/k^T blocks per s-chunk: [p(d, 2 heads), jt, 512]
            # jt: 0,1 = q pairs, 2,3 = k pairs
            qkvT = [persist.tile([128, 4, 512], bf16, name=f"qkvT{sc}", tag=f"qkvT{sc}") for sc in range(SC)]
            # ctx^T blocks per q-chunk: [p(f within pair), pair, 512]
            ctxT = [persist.tile([128, 2, 512], bf16, name=f"ctxT{qc}", tag=f"ctxT{qc}") for qc in range(SC)]
            # v in normal layout + ones column: [p(k within tile), head, kt, 65]
            v_all = persist.tile([128, HEADS_PER_CORE, KT, 65], bf16, tag="v_all")
            # staging for the tail pair-1 out-projection partial
            t1_sb = persist.tile([128, 4, 1024], bf16, tag="t1sb")
            warm_sb = consts.tile([128, 512], bf16, tag="warmup")

            def emit_const_loads():
                # softmax-denominator ones column of the augmented V, and the
                # ones row for the reciprocal partition-broadcast matmul
                nc.vector.memset(v_all[:, :, :, 64:65], 1.0)
                # HAM warm-up: ~3.6us of dependency-free matmuls on a memset
                # tile while the first DMA pieces are in flight, so the PE
                # clock is already at 2.4 GHz when real work arrives (the
                # clock gate needs one full 3.4us window of activity)
                nc.vector.memset(warm_sb[:], 0.0)
                wps = fillP.tile([128, 512], f32, tag="fill", name="hamwarm")
                for r in range(17):
                    nc.tensor.matmul(wps, lhsT=warm_sb[0:128, 0:128],
                                     rhs=warm_sb[:, :],
                                     start=(r == 0), stop=(r == 16))
                # loads share a global ~125 GB/s cap, so only the two
                # queues carrying critical startup data stream early (sync:
                # hT pieces; scalar: wqkv qk pieces then v / strip / wo) and
                # everything else queues behind them.  NOTE: small-packet
                # (512B-1KB run) DMAs empirically outrun 2-4KB-run ones here,
                # so the interleaved wqkv layout is kept deliberately.
                for ht in range(8):
                    nc.scalar.dma_start(out=wqkv_sb[:, ht:ht + 1, 0:512], in_=wqkv[:, ht:ht + 1, 0:512])
                nc.scalar.dma_start(out=wqkv_sb[:, :, 512:768], in_=wqkv[:, :, 512:768])
                if strip_sb is not None:
                    nc.scalar.dma_start(out=strip_sb, in_=strip)

            def load_chunk(sc, eng, pieces=2):
                hT_t = stream.tile([128, 8, 512], bf16, tag="hT", name="hT_t")
                step = 8 // pieces
                for ht in range(0, 8, step):
                    eng.dma_start(out=hT_t[:, ht:ht + step, :],
                                  in_=hT4[sc, :, ht * 512:(ht + step) * 512])
                return hT_t

            # ---- filler units ---------------------------------------------
            def qk_filler(sc, hT_t, jt):
                def f():
                    acc = fillP.tile([128, 512], f32, tag="fill", name="ps")
                    for ht in range(8):
                        nc.tensor.matmul(
                            acc,
                            lhsT=wqkv_sb[:, ht, jt * 128:(jt + 1) * 128],
                            rhs=hT_t[:, ht, :],
                            start=(ht == 0),
                            stop=(ht == 7),
                        )
                    nc.vector.tensor_copy(qkvT[sc][:, jt, :], acc)
                return f

            def v_filler(sc, hT_t, i):
                def f():
                    kt = sc * 4 + i
                    pv = fillP.tile([128, 512], f32, tag="fill", name="pv")
                    for ht in range(8):
                        nc.tensor.matmul(
                            pv[:, 0:256],
                            lhsT=hT_t[:, ht, i * 128:(i + 1) * 128],
                            rhs=wqkv_sb[:, ht, 512:768],
                            start=(ht == 0),
                            stop=(ht == 7),
                        )
                    nc.vector.tensor_copy(v_all[:, :, kt, 0:64], pv[:, 0:256])
                return f

            def next_ring():
                eng = store_rings[0]
                store_rings.append(store_rings.pop(0))
                return eng

            def outproj_filler(blk, i, ec, act_copy=False):
                # full 2-pair unit for blocks 0..2: [128 q rows] x [512 cols]
                def f():
                    st = blk * 4 + i
                    po = fillP.tile([128, 512], f32, tag="fill", name="po")
                    for ft in range(2):
                        nc.tensor.matmul(
                            po,
                            lhsT=ctxT[blk][:, ft, i * 128:(i + 1) * 128],
                            rhs=wo_sb[:, ft, ec * 512:(ec + 1) * 512],
                            start=(ft == 0), stop=(ft == 1),
                        )
                    o_sb = outp.tile([128, 512], bf16, tag="o_sb", name="o_sb")
                    if act_copy:
                        nc.scalar.copy(o_sb, po)
                    else:
                        nc.vector.tensor_copy(o_sb, po)
                    next_ring().dma_start(
                        out=out[st * 128:(st + 1) * 128, ec * 512:(ec + 1) * 512],
                        in_=o_sb)
                return f

            def tail_ft0_filler(blk, i, ec, act_copy=False):
                # pair-0 half-unit of the final block's out-projection,
                # stored as a partial (host adds t0 + t1)
                def f():
                    po = fillP.tile([128, 512], f32, tag="fill", name="pot")
                    nc.tensor.matmul(
                        po,
                        lhsT=ctxT[blk][:, 0, i * 128:(i + 1) * 128],
                        rhs=wo_sb[:, 0, ec * 512:(ec + 1) * 512],
                        start=True, stop=True,
                    )
                    o_sb = outp.tile([128, 512], bf16, tag="o_sb", name="ot_sb")
                    if act_copy:
                        nc.scalar.copy(o_sb, po)
                    else:
                        nc.vector.tensor_copy(o_sb, po)
                    next_ring().dma_start(out=out_t0[i, :, ec * 512:(ec + 1) * 512], in_=o_sb)
                return f

            def qkv_fillers(sc, hT_t):
                return ([qk_filler(sc, hT_t, jt) for jt in range(4)]
                        + [v_filler(sc, hT_t, i) for i in range(4)])

            def emit_attention(qc, fillers, tail_fillers=(), tail_exec=None,
                               tail_warm=(), premade=None):
                # tail_fillers: emitted only during the second pair's k-loop
                # (they depend on pair 0's ctxT of this chunk)
                nkt = n_kt(qc)
                steps = 2 * nkt
                nf = len(fillers)
                due = [(j * steps) // nf for j in range(nf)] if nf else []
                step = 0
                fi = 0
                tfi = 0
                ntf = len(tail_fillers)
                tdue = ([nkt + 1 + (j * (nkt - 2)) // ntf for j in range(ntf)]
                        if ntf else [])
                for pair in range(2):
                    hA, hB = 2 * pair, 2 * pair + 1
                    ctxA = ctxP.tile([65, 512], f32, tag="ctxA", name="ctxA")
                    ctxB = ctxP.tile([65, 512], f32, tag="ctxB", name="ctxB")

                    def emit_av(kt, w0, E):
                        nc.tensor.matmul(
                            ctxA[:, w0:512],
                            lhsT=v_all[:, hA, kt, :],
                            rhs=E[:, 0, w0:512],
                            start=(kt == 0), stop=(kt == nkt - 1),
                        )
                        nc.tensor.matmul(
                            ctxB[:, w0:512],
                            lhsT=v_all[:, hB, kt, :],
                            rhs=E[:, 1, w0:512],
                            start=(kt == 0), stop=(kt == nkt - 1),
                        )

                    # software-pipelined: scores(kt) and exp(kt) issue before
                    # AV(kt-1), so the Scalar engine's exps run back-to-back
                    # and AV never waits on a just-issued exp
                    prev = None
                    for kt in range(nkt):
                        # diagonal tiles only need columns j >= 128*t
                        diag = mode == "causal" and kt >= 4 * qc
                        w0 = 128 * (kt - 4 * qc) if diag else 0
                        if premade is not None and (pair, kt) in premade:
                            # scores+exp for this tile ran during the previous
                            # chunk's boundary; AV can start immediately
                            E = premade[(pair, kt)]
                        else:
                            sp = spP.tile([128, 1024], f32, tag="sp")
                            kblk, ki = qkvT[kt // 4], (kt % 4) * 128
                            # transposed scores, 2 heads row-packed on the PE
                            nc.tensor.matmul(
                                sp[:, w0:512],
                                lhsT=kblk[0:64, 2 + pair, ki:ki + 128],
                                rhs=qkvT[qc][0:64, 0 + pair, w0:512],
                                start=True, stop=True,
                            )
                            nc.tensor.matmul(
                                sp[:, 512 + w0:1024],
                                lhsT=kblk[64:128, 2 + pair, ki:ki + 128],
                                rhs=qkvT[qc][64:128, 0 + pair, w0:512],
                                start=True, stop=True,
                            )
                            E = epool.tile([128, 2, 512], bf16, tag="E")
                            spv = sp[:].rearrange("p (two q) -> p two q", two=2)
                            if kt == 0:
                                # split the first exp per head so AV(kt0, head
                                # A) starts half an exp earlier at boundaries
                                nc.scalar.activation(E[:, 0:1, w0:512], spv[:, 0:1, w0:512], EXP)
                                nc.scalar.activation(E[:, 1:2, w0:512], spv[:, 1:2, w0:512], EXP)
                            else:
                                nc.scalar.activation(E[:, :, w0:512], spv[:, :, w0:512], EXP)
                        if prev is not None:
                            emit_av(*prev)
                        # PE filler absorbs the remaining exp-period slack
                        while fi < nf and due[fi] <= step:
                            fillers[fi]()
                            fi += 1
                        while tfi < ntf and pair == 1 and tdue[tfi] <= nkt + kt:
                            tail_fillers[tfi]()
                            tfi += 1
                        step += 1
                        if diag:
                            # zero the strictly-masked staircase inside the window
                            nc.vector.tensor_mul(
                                E[:, :, w0:512],
                                E[:, :, w0:512],
                                strip_sb[:, :, 384:896 - w0])
                        prev = (kt, w0, E)
                    emit_av(*prev)
                    if tail_exec is not None and pair == 1:
                        for w in tail_warm:
                            w()
                    # normalization: rows 64 hold the softmax denominators.
                    # reciprocal reads PSUM directly; high priority so the
                    # chain doesn't queue behind filler copies
                    if pair == 0:
                        with tc.high_priority(offset=40):
                            rA = norm.tile([1, 512], f32, tag="rA", name="rA")
                            rB = norm.tile([1, 512], f32, tag="rB", name="rB")
                            nc.vector.tensor_copy(rA, ctxA[64:65, :])
                            nc.scalar.copy(rB, ctxB[64:65, :])
                            rr = norm.tile([1, 1024], f32, tag="rr", name="rr")
                            rbc = norm.tile([128, 1024], f32, tag="rbc", name="rbc")
                            nc.vector.reciprocal_approx_fast(out=rr[0:1, 0:512], in_=rA[:])
                            nc.gpsimd.partition_broadcast(rbc[0:64, 0:512], rr[0:1, 0:512], channels=64)
                            nc.vector.reciprocal_approx_fast(out=rr[0:1, 512:1024], in_=rB[:])
                            nc.vector.tensor_mul(ctxT[qc][0:64, pair, :], ctxA[0:64, :], rbc[0:64, 0:512])
                            nc.gpsimd.partition_broadcast(rbc[0:64, 512:1024], rr[0:1, 512:1024], channels=64)
                            nc.vector.tensor_mul(ctxT[qc][64:128, pair, :], ctxB[0:64, :], rbc[0:64, 512:1024])
                    elif tail_exec is None:
                        with tc.high_priority(offset=40):
                            rA = norm.tile([1, 512], f32, tag="rA", name="rA")
                            rB = norm.tile([1, 512], f32, tag="rB", name="rB")
                            nc.vector.tensor_copy(rA, ctxA[64:65, :])
                            nc.scalar.copy(rB, ctxB[64:65, :])
                            rr = norm.tile([1, 1024], f32, tag="rr", name="rr")
                            rbc = norm.tile([128, 1024], f32, tag="rbc", name="rbc")
                            nc.vector.reciprocal_approx_fast(out=rr[0:1, 0:512], in_=rA[:])
                            nc.gpsimd.partition_broadcast(rbc[0:64, 0:512], rr[0:1, 0:512], channels=64)
                            nc.vector.reciprocal_approx_fast(out=rr[0:1, 512:1024], in_=rB[:])
                            nc.vector.tensor_mul(ctxT[qc][0:64, pair, :], ctxA[0:64, :], rbc[0:64, 0:512])
                            nc.gpsimd.partition_broadcast(rbc[0:64, 512:1024], rr[0:1, 512:1024], channels=64)
                            nc.vector.tensor_mul(ctxT[qc][64:128, pair, :], ctxB[0:64, :], rbc[0:64, 512:1024])
                    else:
                        # tail pair-1 norm gates the final out-projection, so
                        # it is sliced by 256 q columns to shorten the chain
                        with tc.high_priority(offset=40):
                            rA = norm.tile([1, 512], f32, tag="rA", name="rA")
                            rB = norm.tile([1, 512], f32, tag="rB", name="rB")
                            rr = norm.tile([1, 1024], f32, tag="rr", name="rr")
                            rbc = norm.tile([128, 1024], f32, tag="rbc", name="rbc")
                            nc.vector.tensor_copy(rA[0:1, 0:256], ctxA[64:65, 0:256])
                            nc.scalar.copy(rB[0:1, 0:256], ctxB[64:65, 0:256])
                            nc.vector.reciprocal_approx_fast(out=rr[0:1, 0:256], in_=rA[0:1, 0:256])
                            nc.gpsimd.partition_broadcast(rbc[0:64, 0:256], rr[0:1, 0:256], channels=64)
                            nc.vector.reciprocal_approx_fast(out=rr[0:1, 512:768], in_=rB[0:1, 0:256])
                            nc.gpsimd.partition_broadcast(rbc[0:64, 512:768], rr[0:1, 512:768], channels=64)
                            nc.vector.tensor_copy(rA[0:1, 256:512], ctxA[64:65, 256:512])
                            nc.scalar.copy(rB[0:1, 256:512], ctxB[64:65, 256:512])
                            nc.vector.tensor_mul(ctxT[qc][0:64, pair, 0:256], ctxA[0:64, 0:256], rbc[0:64, 0:256])
                            nc.vector.tensor_mul(ctxT[qc][64:128, pair, 0:256], ctxB[0:64, 0:256], rbc[0:64, 512:768])
                        if tail_exec is not None:
                            tail_exec(0)
                        with tc.high_priority(offset=40):
                            nc.vector.reciprocal_approx_fast(out=rr[0:1, 256:512], in_=rA[0:1, 256:512])
                            nc.gpsimd.partition_broadcast(rbc[0:64, 256:512], rr[0:1, 256:512], channels=64)
                            nc.vector.reciprocal_approx_fast(out=rr[0:1, 768:1024], in_=rB[0:1, 256:512])
                            nc.gpsimd.partition_broadcast(rbc[0:64, 768:1024], rr[0:1, 768:1024], channels=64)
                            nc.vector.tensor_mul(ctxT[qc][0:64, pair, 256:512], ctxA[0:64, 256:512], rbc[0:64, 256:512])
                            nc.vector.tensor_mul(ctxT[qc][64:128, pair, 256:512], ctxB[0:64, 256:512], rbc[0:64, 768:1024])
                        if tail_exec is not None:
                            tail_exec(1)
                # drain any fillers not yet emitted
                while fi < nf:
                    fillers[fi]()
                    fi += 1
                while tfi < ntf:
                    tail_fillers[tfi]()
                    tfi += 1

            # ---- top-level schedule --------------------------------------
            store_rings[:] = [nc.gpsimd, nc.sync]
            hT_cur = stream.tile([128, 8, 512], bf16, tag="hT", name="hT_t")
            for ht in range(8):
                nc.sync.dma_start(out=hT_cur[:, ht:ht + 1, :],
                                  in_=hT4[0, :, ht * 512:(ht + 1) * 512])
            emit_const_loads()
            hT_next1 = load_chunk(1, nc.gpsimd)
            # gate chunks 2-3 (and wo) behind the v-columns arrival so they
            # don't steal load bandwidth from the critical startup burst
            gate_sb = consts.tile([1, 8], bf16, tag="gate")
            nc.gpsimd.tensor_copy(gate_sb[0:1, 0:8], wqkv_sb[0:1, 0, 512:520])
            hT_next2 = load_chunk(2, nc.gpsimd)
            nc.gpsimd.dma_start(out=wo_sb, in_=wo)

            # chunk-0 qkv, chain-major in DMA-arrival order: the qk chains
            # stream the hT/wqkv pieces as they land (the qk1/qk3 re-runs
            # bridge the wait for the v columns), then the v chains.  Chains
            # spread across three PSUM pools so copies pipeline.
            def chain_qk(jt, acc):
                for ht in range(8):
                    nc.tensor.matmul(
                        acc,
                        lhsT=wqkv_sb[:, ht, jt * 128:(jt + 1) * 128],
                        rhs=hT_cur[:, ht, :],
                        start=(ht == 0), stop=(ht == 7),
                    )
                if jt % 2 == 0:
                    nc.vector.tensor_copy(qkvT[0][:, jt, :], acc)
                else:
                    nc.scalar.copy(qkvT[0][:, jt, :], acc)

            def chain_v(i, acc):
                for ht in range(8):
                    nc.tensor.matmul(
                        acc[:, 0:256],
                        lhsT=hT_cur[:, ht, i * 128:(i + 1) * 128],
                        rhs=wqkv_sb[:, ht, 512:768],
                        start=(ht == 0), stop=(ht == 7),
                    )
                if i % 2 == 0:
                    nc.vector.tensor_copy(v_all[:, :, i, 0:64], acc[:, 0:256])
                else:
                    nc.scalar.copy(v_all[:, :, i, 0:64], acc[:, 0:256])

            # qk chains run as a 4-wide wavefront: every 2-ht DMA piece
            # feeds 8 matmuls (~3.4us cold) versus a ~3us piece interval, so
            # the PE never starves while chunk 0 streams in
            wacc = [fillP.tile([128, 512], f32, tag="fill", name="wa0"),
                    fillP.tile([128, 512], f32, tag="fill", name="wa1"),
                    spP.tile([128, 512], f32, tag="sp", name="wa2"),
                    spP.tile([128, 512], f32, tag="sp", name="wa3")]
            worder = (0, 2, 1, 3)
            for ht in range(8):
                for ci, jt in enumerate(worder):
                    nc.tensor.matmul(
                        wacc[ci],
                        lhsT=wqkv_sb[:, ht, jt * 128:(jt + 1) * 128],
                        rhs=hT_cur[:, ht, :],
                        start=(ht == 0), stop=(ht == 7),
                    )
            for ci, jt in enumerate(worder):
                if ci % 2 == 0:
                    nc.vector.tensor_copy(qkvT[0][:, jt, :], wacc[ci])
                else:
                    nc.scalar.copy(qkvT[0][:, jt, :], wacc[ci])
            chain_v(0, ctxP.tile([128, 512], f32, tag="ctxA", name="wa4"))
            chain_v(1, ctxP.tile([128, 512], f32, tag="ctxB", name="wa5"))
            chain_v(2, fillP.tile([128, 512], f32, tag="fill", name="wa6"))
            chain_v(3, fillP.tile([128, 512], f32, tag="fill", name="wa7"))
            deferred0 = []

            hT_chunks = {0: hT_cur, 1: hT_next1, 2: hT_next2}
            if mode == "causal":
                hT_chunks[3] = load_chunk(3, nc.gpsimd)
                # qc0: rest of qkv(0) + qkv(1); qc1: qkv(2); qc2: qkv(3) +
                # outproj(0); qc3: outproj(1,2) + tail pair-0 preruns
                emit_attention(0, qkv_fillers(1, hT_chunks[1]))
                emit_attention(1, qkv_fillers(2, hT_chunks[2]))
                emit_attention(2, qkv_fillers(3, hT_chunks[3])
                               + [outproj_filler(0, i, ec) for i in range(4) for ec in (0,)])
                # prescore qc3 pair-0 kt0-3 in the boundary window: the PE
                # fills the gap left by qc2's norm chain and the ACT engine
                # gets a 4-tile head start on qc3's exp load
                premade = {}
                for kt in range(4):
                    sp = spP.tile([128, 1024], f32, tag="sp", name="presp")
                    kblk, ki = qkvT[kt // 4], (kt % 4) * 128
                    nc.tensor.matmul(sp[:, 0:512],
                                     lhsT=kblk[0:64, 2, ki:ki + 128],
                                     rhs=qkvT[3][0:64, 0, :],
                                     start=True, stop=True)
                    nc.tensor.matmul(sp[:, 512:1024],
                                     lhsT=kblk[64:128, 2, ki:ki + 128],
                                     rhs=qkvT[3][64:128, 0, :],
                                     start=True, stop=True)
                    Ep = epool.tile([128, 2, 512], bf16, tag="Epre", bufs=4, name=f"Epre{kt}")
                    nc.scalar.activation(
                        Ep[:, :, :],
                        sp[:].rearrange("p (two q) -> p two q", two=2)[:, :, :],
                        EXP)
                    premade[(0, kt)] = Ep
                units = [(i, ec) for i in range(4) for ec in range(2)]
                tail_pre = [tail_ft0_filler(3, i, ec) for (i, ec) in units[:6]]
                op_units = ([outproj_filler(0, i, 1) for i in range(4)]
                            + [outproj_filler(b, i, ec)
                               for b in (1, 2) for i in range(4) for ec in range(2)])
                warm = ([tail_ft0_filler(3, i, ec, act_copy=True) for (i, ec) in units[6:]]
                        + op_units[-2:])

                def tail_exec(s):
                    # ft1 (pair-1) half-units of the final block; copies ride
                    # ACT for slice 0 and DVE for slice 1 so neither engine
                    # serializes the tail
                    for i in (2 * s, 2 * s + 1):
                        for ec in range(2):
                            pool, tag = ((spP, "sp") if (i + ec) % 2 else (fillP, "fill"))
                            po = pool.tile([128, 512], f32, tag=tag, name="pot1")
                            nc.tensor.matmul(
                                po,
                                lhsT=ctxT[3][:, 1, i * 128:(i + 1) * 128],
                                rhs=wo_sb[:, 1, ec * 512:(ec + 1) * 512],
                                start=True, stop=True,
                            )
                            if (i + ec) % 2 == 0:
                                nc.scalar.copy(t1_sb[:, i, ec * 512:(ec + 1) * 512], po)
                            else:
                                nc.vector.tensor_copy(t1_sb[:, i, ec * 512:(ec + 1) * 512], po)
                        next_ring().dma_start(out=out_t1[i], in_=t1_sb[:, i, :])

                emit_attention(3, op_units[:-2],
                               tail_fillers=tail_pre, tail_exec=tail_exec,
                               tail_warm=warm, premade=premade)
            else:
                # full mask: every k-loop reads all chunks' K/V, so ALL
                # qkv chains must complete before attention starts
                hT3 = load_chunk(3, nc.gpsimd)
                for sc, hT_t in ((1, hT_chunks[1]), (2, hT_chunks[2]), (3, hT3)):
                    for f in qkv_fillers(sc, hT_t):
                        f()
                emit_attention(0, [])
                emit_attention(1, [outproj_filler(0, i, ec) for i in range(4) for ec in range(2)])
                emit_attention(2, [outproj_filler(1, i, ec) for i in range(4) for ec in range(2)])
                units = [(i, ec) for i in range(4) for ec in range(2)]
                tail_pre = [tail_ft0_filler(3, i, ec) for (i, ec) in units[:6]]
                op_units = [outproj_filler(2, i, ec) for i in range(4) for ec in range(2)]
                warm = ([tail_ft0_filler(3, i, ec, act_copy=True) for (i, ec) in units[6:]]
                        + op_units[-2:])

                def tail_exec(s):
                    # ft1 (pair-1) half-units of the final block; copies ride
                    # ACT for slice 0 and DVE for slice 1 so neither engine
                    # serializes the tail
                    for i in (2 * s, 2 * s + 1):
                        for ec in range(2):
                            pool, tag = ((spP, "sp") if (i + ec) % 2 else (fillP, "fill"))
                            po = pool.tile([128, 512], f32, tag=tag, name="pot1")
                            nc.tensor.matmul(
                                po,
                                lhsT=ctxT[3][:, 1, i * 128:(i + 1) * 128],
                                rhs=wo_sb[:, 1, ec * 512:(ec + 1) * 512],
                                start=True, stop=True,
                            )
                            if (i + ec) % 2 == 0:
                                nc.scalar.copy(t1_sb[:, i, ec * 512:(ec + 1) * 512], po)
                            else:
                                nc.vector.tensor_copy(t1_sb[:, i, ec * 512:(ec + 1) * 512], po)
                        next_ring().dma_start(out=out_t1[i], in_=t1_sb[:, i, :])

                emit_attention(3, op_units[:-2],
                               tail_fillers=tail_pre, tail_exec=tail_exec,
                               tail_warm=warm)

    nc.compile()
    return nc


def _get_program(mode):
    if mode not in _CACHE:
        _CACHE[mode] = _build(mode)
    return _CACHE[mode]


def _classify_mask(mask):
    """Return "causal", "full", or "generic"."""
    m = mask.reshape(B, S, S)
    tril = np.tril_indices(S)
    if np.all(m == 0.0):
        return "full"
    for b in range(B):
        mb = m[b]
        if not np.all(mb[tril] == 0.0):
            return "generic"
        if not np.all(mb[np.triu_indices(S, k=1)] < -240.0):
            return "generic"
    return "causal"


def _prepare_in_maps(hidden_states, w_qkv, w_out):
    import concourse.mybir as mybir

    bf16 = mybir.dt.np(mybir.dt.bfloat16)

    # strip[i, d, x] = 1.0 iff x >= i + 384 (duplicated along d for head pairs)
    base = (np.arange(896, dtype=np.int32)[None, :] >= (np.arange(128, dtype=np.int32)[:, None] + 384)).astype(np.float32)
    strip = np.ascontiguousarray(np.broadcast_to(base[:, None, :], (128, 2, 896))).astype(bf16)

    # hT4[sc, p, ht*512 + s'] = hs[b, sc*512 + s', ht*128 + p]
    hT4 = [np.ascontiguousarray(
        hidden_states[b].reshape(SC, 512, 8, 128).transpose(0, 3, 2, 1).reshape(SC, 128, 8 * 512)
    ).astype(bf16) for b in range(B)]

    in_maps = []
    for c in range(N_CORES):
        b, g = divmod(c, 4)
        cols = []
        for part in (0, 1):  # q, k column groups of w_qkv (pair-packed)
            for pair in range(2):
                for h in (4 * g + 2 * pair, 4 * g + 2 * pair + 1):
                    cols.append(w_qkv[:, part * H + h * DH: part * H + (h + 1) * DH])
        for h in range(4 * g, 4 * g + 4):  # v columns, head-major
            cols.append(w_qkv[:, 2 * H + h * DH: 2 * H + (h + 1) * DH])
        wqkv_c = np.concatenate(cols, axis=1)
        # fold the 1/sqrt(DH) score scale into the q columns
        wqkv_c = np.ascontiguousarray(wqkv_c)
        wqkv_c[:, 0:256] *= 1.0 / np.sqrt(DH)
        # [p, ht, j] layout
        wqkv_c = wqkv_c.reshape(8, 128, 768).transpose(1, 0, 2)
        wo_c = w_out[g * F_CORE:(g + 1) * F_CORE, :].reshape(2, 128, H).transpose(1, 0, 2)
        in_maps.append({"hT4": hT4[b],
                        "wqkv": np.ascontiguousarray(wqkv_c).astype(bf16),
                        "wo": np.ascontiguousarray(wo_c).astype(bf16),
                        "strip": strip})
    return in_maps


def _run(inputs, trace=False):
    from concourse.bass_utils import run_bass_kernel_spmd

    hidden_states = np.asarray(inputs["hidden_states"], dtype=np.float32)
    mask = np.asarray(inputs["attention_mask"], dtype=np.float32)
    w_qkv = np.asarray(inputs["w_qkv"], dtype=np.float32)
    w_out = np.asarray(inputs["w_out"], dtype=np.float32)

    mode = _classify_mask(mask)
    if mode == "generic":
        return _numpy_reference(hidden_states, mask, w_qkv, w_out), None

    nc = _get_program(mode)
    in_maps = _prepare_in_maps(hidden_states, w_qkv, w_out)
    res = run_bass_kernel_spmd(nc, in_maps, list(range(N_CORES)), trace=trace)
    out = np.zeros((B, S, H), dtype=np.float32)
    for c in range(N_CORES):
        r = res.results[c]
        out[c // 4][:3 * 512] += r["out"].astype(np.float32)
        tail = r["out_t0"].astype(np.float32) + r["out_t1"].astype(np.float32)
        out[c // 4][3 * 512:] += tail.reshape(512, H)
    return out, res


def kernel(**inputs):
    out, _ = _run(inputs, trace=False)
    return out


def kernel_traced(**inputs):
    """Like kernel() but with NTFF profiling; returns (out, BassKernelResults)."""
    return _run(inputs, trace=True)


def _numpy_reference(hidden_states, mask, w_qkv, w_out):
    """Exact fallback for unrecognized masks (slow, chunked numpy)."""
    out = np.zeros((B, S, H), dtype=np.float32)
    m = mask.reshape(B, 1, S, S)
    for b in range(B):
        qkv = hidden_states[b] @ w_qkv  # [S, 3H]
        q = qkv[:, 0:H].reshape(S, NH, DH)
        k = qkv[:, H:2 * H].reshape(S, NH, DH)
        v = qkv[:, 2 * H:].reshape(S, NH, DH)
        ctx = np.zeros((S, NH, DH), dtype=np.float32)
        for h in range(NH):
            s = (q[:, h] @ k[:, h].T) / np.sqrt(DH) + m[b, 0]
            s = s - s.max(axis=-1, keepdims=True)
            e = np.exp(s)
            p = e / e.sum(axis=-1, keepdims=True)
            ctx[:, h] = p @ v[:, h]
        out[b] = ctx.reshape(S, H) @ w_out
    return out
